# revision 1
# baseline (speedup 1.0000x reference)
"""Channel-attention (per-head [64,64] score matrix) Trainium2 Bass kernel.

Math (per batch b of 16):
    qkv = x @ w_qkv                 # x [4096, 256], w_qkv [256, 1536]
    q,k,v = split(qkv); per head h (8 heads x 64 dim):
    sim_h = (q_h * 8^-1)^T @ k_h    # [64, 64]   (contracts spatial d=4096)
    attn_h = softmax(sim_h, axis=-1)
    out_h = v_h @ attn_h^T          # [4096, 64]
    y = concat(out_h) @ w_out + b_out

Distribution: data-parallel over batch — 8 cores x 2 batches each; weights
replicated; no collectives. The host pre-transposes x to [C, d] per batch so
every device matmul streams with large free dims, pre-folds the 1/8 q-scale
into w_q, pre-converts inputs to fp16 (all matmuls run fp16 x fp16 with fp32
PSUM accumulation; end-to-end rel-l2 ~1.6e-3 vs fp64 oracle), and adds the
output bias on the host (so y can DMA straight out of PSUM).

Device dataflow per batch (phases ordered so V-phase matmuls hide the
softmax latency on PE):
  QK:   q,k [d-chunk 128, 512each] (lhsT = xT d-chunk, rhs = w_qk cols, N=512)
  B:    sim[p] [128,128] per head-pair accumulates over 32 d-chunks
  V:    vT[m,d] = w_v.T @ xT       (lhsT = w_v chunk, rhs = xT d-cols, N=512)
  soft: rowmax (negated) -> exp(sim - max) with accum_out row-sums ->
        recip -> scale e rows by 1/s (so C1's PSUM drain is a plain copy)
  T:    PE-transpose e_p -> eT_p (C1's stationary operand)
  C1:   outT[i,d] = eT_h @ vT_h, two heads per PE pass (row/col split)
  C2:   y[d,c] = outT.T @ w_out, DMA'd to HBM directly from PSUM (fp32)
"""

import numpy as np

import concourse.bass as bass
import concourse.mybir as mybir
from concourse.bass_utils import run_bass_kernel_spmd
from concourse.masks import make_identity
from concourse.tile import TileContext


def _split_multi_waits(nc, limit=1):
    """Post-pass: the walrus build in this container rejects instructions
    carrying more than `limit` sync-waits ("Too many sync wait commands" in
    setupSyncWait). Tile attaches up to 3. Hoist the extras onto same-engine
    NoOp instructions inserted immediately before the owner — the engine
    sequencer executes them in order, so the ordering semantics are
    identical (single-wait instructions are what the rest of the Tile
    output uses, and those compile)."""
    drain_engines = [
        mybir.EngineType.PE,
        mybir.EngineType.DVE,
        mybir.EngineType.Activation,
        mybir.EngineType.Pool,
        mybir.EngineType.SP,
    ]
    n_split = 0
    for f in nc.m.functions:
        for blk in f.blocks:
            il = blk.instructions
            i = 0
            while i < len(il):
                inst = il[i]
                si = inst.sync_info
                waits = list(si.on_wait) if si is not None else []
                if len(waits) > limit:
                    si.on_wait = waits[:limit]
                    # The kernel-tail drain aggregates one wait per logical
                    # processor; those can wait in parallel across engines
                    # (the all-engine barrier that follows orders them before
                    # the semaphore clears). Mid-program instructions keep
                    # their extras on their own engine to preserve ordering.
                    is_drain = type(inst).__name__ == "InstDrain"
                    for k, w in enumerate(waits[limit:]):
                        nop = mybir.InstNoOp(
                            name=f"I-waitsplit-{n_split}", ins=[], outs=[]
                        )
                        n_split += 1
                        nop.engine = (
                            drain_engines[k % len(drain_engines)]
                            if is_drain else inst.engine
                        )
                        nop.sync_info = mybir.SyncInfo(on_wait=[w], on_update=[])
                        il.insert(i, nop)
                        i += 1
                i += 1
    return nc


N_CORES = 8
BATCH = 16
BPC = BATCH // N_CORES  # batches per core
D = 4096  # spatial (64*64)
C = 256   # channels
HID = 512
HEADS = 8
DH = 64

F32 = mybir.dt.float32
F16 = mybir.dt.float16

_CACHE = {}


def _build():
    nc = bass.Bass()
    xT_d = nc.declare_dram_parameter("xT", [BPC, C, D], F16, isOutput=False)
    wqkv_d = nc.declare_dram_parameter("w_qkv", [C, 3 * HID], F16, isOutput=False)
    wout_d = nc.declare_dram_parameter("w_out_r", [128, 4, C], F16, isOutput=False)
    y_d = nc.declare_dram_parameter("y", [BPC, D, C], F32, isOutput=True)

    with TileContext(nc) as tc:
        with (
            tc.tile_pool(name="consts", bufs=1) as consts,
            tc.tile_pool(name="xt", bufs=2) as xt_pool,
            tc.tile_pool(name="vt", bufs=8) as vt_pool,
            tc.tile_pool(name="qk", bufs=6) as qk_pool,
            tc.tile_pool(name="eP", bufs=8) as e_pool,
            tc.tile_pool(name="stat", bufs=6) as stat_pool,
            tc.tile_pool(name="ot", bufs=12) as ot_pool,
            tc.tile_pool(name="ysb", bufs=8) as y_pool,
            tc.tile_pool(name="mm", bufs=6, space="PSUM") as mm_pool,
            tc.tile_pool(name="simp", bufs=2, space="PSUM") as sim_pool,
        ):
            # ---- constants ----
            # w_qkv split loads ordered by first use: w_q, then w_k, then
            # w_v / w_out (V and C2 run much later).
            w_sb = []
            for ci in range(2):
                w_t = consts.tile([128, 3 * HID], F16, name=f"w{ci}")
                w_sb.append(w_t)
            for ci in range(2):
                nc.sync.dma_start(
                    out=w_sb[ci][:, 0:HID],
                    in_=wqkv_d[ci * 128:(ci + 1) * 128, 0:HID],
                )
            wo_sb = consts.tile([128, 4, C], F16, name="wo")
            ident = consts.tile([128, 128], F32, name="ident")
            make_identity(nc, ident)

            for b in range(BPC):
                # ---- load xT (chunked so the first QK matmuls start early) --
                xt = []
                for ci in range(2):
                    x_t = xt_pool.tile([128, D], F16, name=f"xt{ci}", tag="xt")
                    xt.append(x_t)
                # first 512 cols arrive alone so QK d1=0..3 can start
                # early; w_k loads are interleaved after them (the k matmuls
                # trail the q matmuls by the pipeline skew anyway)
                chunks = [(0, 512)] + [(lo, lo + 896) for lo in range(512, D, 896)]
                for ki, (lo, hi) in enumerate(chunks):
                    hi = min(hi, D)
                    for ci in range(2):
                        nc.sync.dma_start(
                            out=xt[ci][:, lo:hi],
                            in_=xT_d[b, ci * 128:(ci + 1) * 128, lo:hi],
                        )
                    if b == 0 and ki == 0:
                        for ci in range(2):
                            nc.sync.dma_start(
                                out=w_sb[ci][:, HID:2 * HID],
                                in_=wqkv_d[ci * 128:(ci + 1) * 128, HID:2 * HID],
                            )

                # ---- phase QK + B ----
                # sim[p]: one PSUM bank per accumulation group (start=True
                # zeroes a whole 2KB zero-region per written partition, so
                # groups must not share a bank). Tile p = head pair
                # (2p, 2p+1): rows i (head 2p at 0:64, 2p+1 at 64:128),
                # cols j likewise; diag 64x64 blocks are the per-head sims.
                # sim_all [128, 256]: ONE psum bank holds all 8 per-head
                # accumulators — pair p at cols p*64:+64, head 2p at rows
                # 0:64, head 2p+1 at rows 64:128. The bank is zeroed by an
                # explicit memset and every matmul uses start=False
                # (accumulate) — order-independent, so the scheduler may
                # interleave the groups freely.
                sim_all = sim_pool.tile([128, 256], F32, name="sim_all", tag="simp")
                nc.vector.memset(sim_all, 0.0)
                def emit_b(qk_tile, d1):
                    # sim matmuls for the qk tile of iteration d1 (emitted one
                    # iteration late so the PSUM->SBUF copy latency hides
                    # under the next iteration's qk matmuls)
                    for p in range(4):
                        for par in range(2):
                            q_lo = p * 128 + par * 64
                            nc.tensor.matmul(
                                sim_all[par * 64:(par + 1) * 64, p * 64:(p + 1) * 64],
                                lhsT=qk_tile[:, q_lo:q_lo + 64],
                                rhs=qk_tile[:, 512 + q_lo:512 + q_lo + 64],
                                start=False,
                                stop=(d1 == 31),
                                skip_group_check=True,
                            )

                prev = None
                for d1 in range(32):
                    qps = mm_pool.tile([128, 512], F32, name="qps", tag="mm")
                    kps = mm_pool.tile([128, 512], F32, name="kps", tag="mm")
                    for ci in range(2):
                        nc.tensor.matmul(
                            qps,
                            lhsT=xt[ci][:, d1 * 128:(d1 + 1) * 128],
                            rhs=w_sb[ci][:, 0:HID],
                            start=(ci == 0),
                            stop=(ci == 1),
                        )
                    for ci in range(2):
                        nc.tensor.matmul(
                            kps,
                            lhsT=xt[ci][:, d1 * 128:(d1 + 1) * 128],
                            rhs=w_sb[ci][:, HID:2 * HID],
                            start=(ci == 0),
                            stop=(ci == 1),
                        )
                    qk = qk_pool.tile([128, 1024], F16, name="qk", tag="qk")
                    nc.any.tensor_copy(qk[:, 0:512], qps)
                    nc.any.tensor_copy(qk[:, 512:1024], kps)
                    if prev is not None:
                        emit_b(*prev)
                    prev = (qk, d1)

                # ---- phase V (PE work that hides softmax latency) ----
                # d5-outer so vt[0..3] become ready column-range by
                # column-range — C1's d5 loop can start at d5=0 early. The
                # first d5 iteration is emitted BEFORE the last deferred B
                # matmuls so the scheduler has PE work to cover the final
                # qk copy's latency.
                if b == 0:
                    # deferred weight loads (not needed until now)
                    for ci in range(2):
                        nc.sync.dma_start(
                            out=w_sb[ci][:, 2 * HID:3 * HID],
                            in_=wqkv_d[ci * 128:(ci + 1) * 128, 2 * HID:3 * HID],
                        )
                    nc.sync.dma_start(out=wo_sb, in_=wout_d[:, :, :])
                vt = []
                for m in range(4):
                    v_t = vt_pool.tile([128, D], F16, name=f"vt{m}", tag="vt")
                    vt.append(v_t)

                def emit_v(d5):
                    for m in range(4):
                        wv_lo = 2 * HID + m * 128
                        vps = mm_pool.tile([128, 512], F32, name="vps", tag="mm")
                        for ci in range(2):
                            nc.tensor.matmul(
                                vps,
                                lhsT=w_sb[ci][:, wv_lo:wv_lo + 128],
                                rhs=xt[ci][:, d5 * 512:(d5 + 1) * 512],
                                start=(ci == 0),
                                stop=(ci == 1),
                            )
                        nc.any.tensor_copy(vt[m][:, d5 * 512:(d5 + 1) * 512], vps)

                emit_b(*prev)
                for d5 in range(8):
                    emit_v(d5)

                # ---- softmax (DVE/ACT; overlaps V on PE) ----
                # head h: pair p=h//2, par=h%2; diag block of sim[p] at
                # rows/cols par*64:+64.
                m_t = stat_pool.tile([128, 4], F32, name="m_t", tag="stat")
                s_t = stat_pool.tile([128, 4], F32, name="s_t", tag="stat")
                r_t = stat_pool.tile([128, 4], F32, name="r_t", tag="stat")
                e_tiles = []
                for p in range(4):
                    e_p = e_pool.tile([128, 128], F32, name=f"e{p}", tag="e")
                    nc.gpsimd.memset(e_p, 0.0)
                    e_tiles.append(e_p)
                for h in range(HEADS):
                    par, p = h % 2, h // 2
                    rows = slice(par * 64, par * 64 + 64)
                    nc.vector.reduce_max(
                        out=m_t[rows, p:p + 1],
                        in_=sim_all[rows, p * 64:(p + 1) * 64],
                        axis=mybir.AxisListType.X,
                        negate=True,
                    )
                for h in range(HEADS):
                    par, p = h % 2, h // 2
                    rows = slice(par * 64, par * 64 + 64)
                    nc.scalar.activation(
                        out=e_tiles[p][rows, par * 64:par * 64 + 64],
                        in_=sim_all[rows, p * 64:(p + 1) * 64],
                        func=mybir.ActivationFunctionType.Exp,
                        bias=m_t[rows, p:p + 1],
                        scale=1.0,
                        accum_out=s_t[rows, p:p + 1],
                    )
                nc.vector.reciprocal(r_t, s_t)
                # attn = e / s: fold 1/s into e rows now (tiny [128,128]
                # tiles) instead of scaling every [128,512] C1 output.
                for p in range(4):
                    nc.vector.tensor_scalar_mul(
                        e_tiles[p], e_tiles[p], r_t[:, p:p + 1]
                    )

                # ---- transpose e -> eT (PE) ----
                eT_tiles = []
                for p in range(4):
                    etps = mm_pool.tile([128, 128], F32, name="etps", tag="mm")
                    nc.tensor.transpose(etps, e_tiles[p], ident)
                    eT_s = e_pool.tile([128, 128], F16, name=f"eT{p}", tag="eT")
                    nc.any.tensor_copy(eT_s, etps)
                    eT_tiles.append(eT_s)

                # ---- phase C: attention-apply + output projection ----
                def emit_c2(ot_tiles, d5):
                    # C2 for d5's ot tiles (emitted one d5 late so the ot
                    # copy latency hides under the next d5's C1 matmuls)
                    for d1 in range(4):
                        yps = mm_pool.tile([128, C], F32, name="yps", tag="mm")
                        for p4 in range(4):
                            nc.tensor.matmul(
                                yps,
                                lhsT=ot_tiles[p4][:, d1 * 128:(d1 + 1) * 128],
                                rhs=wo_sb[:, p4, :],
                                start=(p4 == 0),
                                stop=(p4 == 3),
                            )
                        ysb = y_pool.tile([128, C], F32, name="ysb", tag="ysb")
                        nc.any.tensor_copy(ysb, yps)
                        d_lo = d5 * 512 + d1 * 128
                        nc.sync.dma_start(out=y_d[b, d_lo:d_lo + 128, :], in_=ysb)

                prev_c = None
                for d5 in range(8):
                    ot_tiles = []
                    for p in range(4):
                        c1ps = mm_pool.tile([128, 512], F32, name="c1ps", tag="mm")
                        # eT_p is exactly block-diagonal (off-diag blocks are
                        # memset zeros), so one full-array K=128 matmul
                        # computes both heads: rows 0:64 of eT only meet
                        # vt rows 0:64 (head 2p), rows 64:128 only head 2p+1.
                        nc.tensor.matmul(
                            c1ps,
                            lhsT=eT_tiles[p],
                            rhs=vt[p][:, d5 * 512:(d5 + 1) * 512],
                            start=True,
                            stop=True,
                        )
                        ot = ot_pool.tile([128, 512], F16, name=f"ot{p}", tag="ot")
                        nc.any.tensor_copy(ot, c1ps)
                        ot_tiles.append(ot)
                    if prev_c is not None:
                        emit_c2(*prev_c)
                    prev_c = (ot_tiles, d5)
                emit_c2(*prev_c)
    return _split_multi_waits(nc)


def _get_nc():
    if "nc" not in _CACHE:
        _CACHE["nc"] = _build()
    return _CACHE["nc"]


def kernel(x, w_qkv, w_out, b_out, **kw):
    x = np.asarray(x, dtype=np.float32)
    w_qkv = np.asarray(w_qkv, dtype=np.float32)
    w_out = np.asarray(w_out, dtype=np.float32)
    b_out = np.asarray(b_out, dtype=np.float32)

    # fold q-scale into w_q (exact: power-of-two scale), then fp16-quantize
    w_qkv_s = w_qkv.copy()
    w_qkv_s[:, :HID] *= DH ** (-0.5)
    w_qkv_s = np.ascontiguousarray(w_qkv_s.astype(np.float16))
    # w_out [512, 256] -> [128, 4, 256] with [p, t, c] = w_out[t*128+p, c]
    w_out_r = np.ascontiguousarray(
        w_out.reshape(4, 128, C).transpose(1, 0, 2).astype(np.float16)
    )

    x4 = x.reshape(BATCH, D, C).astype(np.float16)
    in_maps = []
    for core in range(N_CORES):
        xs = np.ascontiguousarray(
            x4[core * BPC:(core + 1) * BPC].transpose(0, 2, 1)
        )  # [BPC, C, D] fp16
        in_maps.append({"xT": xs, "w_qkv": w_qkv_s, "w_out_r": w_out_r})

    nc = _get_nc()
    res = run_bass_kernel_spmd(nc, in_maps, core_ids=list(range(N_CORES)), **kw)
    y = np.concatenate([r["y"] for r in res.results], axis=0)  # [16, 4096, 256]
    y += b_out  # bias on host (broadcast over last axis)
    return y.reshape(BATCH, 64, 64, C)



# revision 3
# speedup vs baseline: 1.9957x; 1.9957x over previous
"""Channel-attention Trainium2 Bass kernel, Gram-collapsed formulation.

Key identity: this is CHANNEL attention (the softmax mixes the 64 channels
of each head; every pixel is treated identically), so the whole module
collapses to a per-batch 256x256 effective channel-mixing matrix:

    G    = x^T x                      # [256,256] Gram, contracts d=4096
    sim_h = wq_h^T G wk_h             # [64,64] per head  (== (x wq)^T (x wk))
    attn_h = softmax(sim_h)
    M_h  = attn_h^T wo_h              # [64,256]
    W    = wv @ concat_h(M_h)         # [256,256] effective weight
    y    = x @ W (+ b_out)

Only G and y touch the [4096, 256] data; everything else is tiny. Per-batch
PE cost ~39k col-cycles vs ~164k for the direct qkv formulation.

Distribution: data-parallel over batch - 8 cores x 2 batches each, weights
replicated, no collectives. Host supplies x twice in fp16 (d-major for G's
d-contraction, c-major for y's c-contraction), pre-folds the 1/8 q-scale
into w_q, and adds bias on the host. All matmuls fp16 x fp16 with fp32 PSUM
accumulation (end-to-end rel-l2 ~2.4e-3 vs fp64 oracle). y returns fp16.

Emission order interleaves the two batches so G(b1) keeps PE busy while
softmax(b0) runs on DVE/ACT, and xdc(b1) is DMA'd before xT(b0) so G(b1)
can start as early as possible.
"""

import numpy as np

import concourse.bass as bass
import concourse.mybir as mybir
from concourse.bass_utils import run_bass_kernel_spmd
from concourse.tile import TileContext


def _split_multi_waits(nc, limit=1):
    """Post-pass: the walrus build in this container rejects instructions
    carrying more than `limit` sync-waits ("Too many sync wait commands" in
    setupSyncWait). Tile attaches up to 3. Hoist the extras onto same-engine
    NoOp instructions inserted immediately before the owner — the engine
    sequencer executes them in order, so the ordering semantics are
    identical."""
    drain_engines = [
        mybir.EngineType.PE,
        mybir.EngineType.DVE,
        mybir.EngineType.Activation,
        mybir.EngineType.Pool,
        mybir.EngineType.SP,
    ]
    n_split = 0
    for f in nc.m.functions:
        for blk in f.blocks:
            il = blk.instructions
            i = 0
            while i < len(il):
                inst = il[i]
                si = inst.sync_info
                waits = list(si.on_wait) if si is not None else []
                if len(waits) > limit:
                    si.on_wait = waits[:limit]
                    is_drain = type(inst).__name__ == "InstDrain"
                    for k, w in enumerate(waits[limit:]):
                        nop = mybir.InstNoOp(
                            name=f"I-waitsplit-{n_split}", ins=[], outs=[]
                        )
                        n_split += 1
                        nop.engine = (
                            drain_engines[k % len(drain_engines)]
                            if is_drain else inst.engine
                        )
                        nop.sync_info = mybir.SyncInfo(on_wait=[w], on_update=[])
                        il.insert(i, nop)
                        i += 1
                i += 1
    return nc


N_CORES = 8
BATCH = 16
BPC = BATCH // N_CORES  # batches per core
D = 4096  # spatial (64*64)
C = 256   # channels
HID = 512
HEADS = 8

F32 = mybir.dt.float32
F16 = mybir.dt.float16

_CACHE = {}


def _build():
    nc = bass.Bass()
    # x twice: d-major (partition = d%128) for G, c-major for Y
    xdc_d = nc.declare_dram_parameter("x_dc", [BPC, 128, 32 * C], F16, isOutput=False)
    xt_d = nc.declare_dram_parameter("xT", [BPC, 128, 2 * D], F16, isOutput=False)
    wq_d = nc.declare_dram_parameter("wq_r", [128, 2, HID], F16, isOutput=False)
    wk_d = nc.declare_dram_parameter("wk_r", [128, 2, HID], F16, isOutput=False)
    wvT_d = nc.declare_dram_parameter("wvT_r", [128, 4, C], F16, isOutput=False)
    wo_d = nc.declare_dram_parameter("wo_r", [128, 4, C], F16, isOutput=False)
    y_d = nc.declare_dram_parameter("y", [BPC, D, C], F16, isOutput=True)

    with TileContext(nc) as tc:
        with (
            tc.tile_pool(name="consts", bufs=1) as consts,
            tc.tile_pool(name="xdc", bufs=2) as xdc_pool,
            tc.tile_pool(name="xt", bufs=2) as xt_pool,
            tc.tile_pool(name="small", bufs=2) as small_pool,
            tc.tile_pool(name="e16", bufs=8) as e_pool,
            tc.tile_pool(name="stat", bufs=6) as stat_pool,
            tc.tile_pool(name="ysb", bufs=8) as y_pool,
            tc.tile_pool(name="mm", bufs=6, space="PSUM") as mm_pool,
            tc.tile_pool(name="simp", bufs=2, space="PSUM") as sim_pool,
        ):
            # ---- weight loads (wq/wk needed first for t2+sim) ----
            wk_sb = consts.tile([128, 2, HID], F16, name="wk")
            wq_sb = consts.tile([128, 2, HID], F16, name="wq")
            wvT_sb = consts.tile([128, 4, C], F16, name="wvT")
            wo_sb = consts.tile([128, 4, C], F16, name="wo")
            nc.sync.dma_start(out=wk_sb, in_=wk_d[:, :, :])
            nc.sync.dma_start(out=wq_sb, in_=wq_d[:, :, :])

            # ---- x DMAs: xdc(b0) -> xdc(b1) -> xT(b0) -> w -> xT(b1) ----
            xdc = []
            xt = []
            for b in range(BPC):
                x_t = xdc_pool.tile([128, 32, C], F16, name=f"xdc{b}", tag="xdc")
                xdc.append(x_t)
                t_t = xt_pool.tile([128, 2, D], F16, name=f"xt{b}", tag="xt")
                xt.append(t_t)
            for b in range(BPC):
                for ci in range(4):  # 8 d-chunks per DMA -> [128, 4KB]
                    lo = ci * 8 * C
                    nc.sync.dma_start(
                        out=xdc[b][:, ci * 8:(ci + 1) * 8, :],
                        in_=xdc_d[b, :, lo:lo + 8 * C],
                    )
            for ci in range(4):  # [128, 4KB] chunks of xT(b0)
                t, lo = divmod(ci * 2048, D)
                nc.sync.dma_start(
                    out=xt[0][:, t, lo:lo + 2048],
                    in_=xt_d[0, :, t * D + lo:t * D + lo + 2048],
                )
            nc.sync.dma_start(out=wvT_sb, in_=wvT_d[:, :, :])
            nc.sync.dma_start(out=wo_sb, in_=wo_d[:, :, :])
            for ci in range(4):
                t, lo = divmod(ci * 2048, D)
                nc.sync.dma_start(
                    out=xt[1][:, t, lo:lo + 2048],
                    in_=xt_d[1, :, t * D + lo:t * D + lo + 2048],
                )

            # ---- per-batch small phases, emitted as closures ----
            def emit_g(b):
                """G = x^T x accumulated over 32 d-chunks; -> Gsb fp16."""
                gps = [
                    mm_pool.tile([128, HID], F32, name=f"gps{m}", tag="mm")
                    for m in range(2)
                ]
                for k in range(32):
                    for m in range(2):
                        nc.tensor.matmul(
                            gps[m][:, 0:C],
                            lhsT=xdc[b][:, k, m * 128:(m + 1) * 128],
                            rhs=xdc[b][:, k, :],
                            start=(k == 0),
                            stop=(k == 31),
                        )
                g_sb = small_pool.tile([128, 2, C], F16, name="gsb", tag="gsb")
                for m in range(2):
                    nc.any.tensor_copy(g_sb[:, m, :], gps[m][:, 0:C])
                return g_sb

            def emit_t2_sim(b, g_sb):
                """t2 = G @ wk -> fp16; sim_h = wq_h^T t2_h -> PSUM."""
                t2_sb = small_pool.tile([128, 2, HID], F16, name="t2sb", tag="t2")
                for m in range(2):
                    t2p = mm_pool.tile([128, HID], F32, name="t2p", tag="mm")
                    for j in range(2):
                        # G symmetric: tile j of Gsb == [c2-chunk j rows, c1]
                        nc.tensor.matmul(
                            t2p,
                            lhsT=g_sb[:, j, m * 128:(m + 1) * 128],
                            rhs=wk_sb[:, j, :],
                            start=(j == 0),
                            stop=(j == 1),
                        )
                    nc.any.tensor_copy(t2_sb[:, m, :], t2p)
                sim_all = sim_pool.tile([128, 2 * C], F32, name="sim", tag="simp")
                nc.vector.memset(sim_all, 0.0)
                for h in range(HEADS):
                    par, p = h % 2, h // 2
                    rows = slice(par * 64, par * 64 + 64)
                    for j in range(2):
                        nc.tensor.matmul(
                            sim_all[rows, p * 64:(p + 1) * 64],
                            lhsT=wq_sb[:, j, h * 64:(h + 1) * 64],
                            rhs=t2_sb[:, j, h * 64:(h + 1) * 64],
                            start=False,
                            stop=(j == 1),
                            skip_group_check=True,
                        )
                return sim_all

            def emit_softmax(b, sim_all):
                """softmax over sim blocks -> block-diag e16 tiles (fp16)."""
                m_t = stat_pool.tile([128, 4], F32, name="m_t", tag="stat")
                s_t = stat_pool.tile([128, 4], F32, name="s_t", tag="stat")
                r_t = stat_pool.tile([128, 4], F32, name="r_t", tag="stat")
                e32, e16 = [], []
                for p in range(4):
                    e_p = e_pool.tile([128, 128], F32, name=f"e32_{p}", tag="e32")
                    # exp writes only diag blocks; the e16 = e32 * r pass
                    # reads the whole tile, so zero the off-diag here
                    nc.gpsimd.memset(e_p, 0.0)
                    e32.append(e_p)
                    f_p = e_pool.tile([128, 128], F16, name=f"e16_{p}", tag="e16")
                    e16.append(f_p)
                for h in range(HEADS):
                    par, p = h % 2, h // 2
                    rows = slice(par * 64, par * 64 + 64)
                    nc.vector.reduce_max(
                        out=m_t[rows, p:p + 1],
                        in_=sim_all[rows, p * 64:(p + 1) * 64],
                        axis=mybir.AxisListType.X,
                        negate=True,
                    )
                for h in range(HEADS):
                    par, p = h % 2, h // 2
                    rows = slice(par * 64, par * 64 + 64)
                    nc.scalar.activation(
                        out=e32[p][rows, par * 64:par * 64 + 64],
                        in_=sim_all[rows, p * 64:(p + 1) * 64],
                        func=mybir.ActivationFunctionType.Exp,
                        bias=m_t[rows, p:p + 1],
                        scale=1.0,
                        accum_out=s_t[rows, p:p + 1],
                    )
                nc.vector.reciprocal(r_t, s_t)
                for p in range(4):
                    # writes only the diag blocks (off-diag stays memset-0):
                    # rows 0:64 -> cols 0:64 (head 2p), rows 64:128 -> cols
                    # 64:128 (head 2p+1) both live in e32[p] diag already
                    nc.vector.tensor_scalar_mul(
                        e16[p], e32[p], r_t[:, p:p + 1]
                    )
                return e16

            def emit_m_weff(b, e16):
                """M_h = attn_h^T wo_h; W = wv @ M -> Wsb fp16."""
                m_sb = small_pool.tile([128, 4, C], F16, name="msb", tag="msb")
                for p in range(4):
                    mp = mm_pool.tile([128, HID], F32, name="mp", tag="mm")
                    nc.tensor.matmul(
                        mp[:, 0:C],
                        lhsT=e16[p],
                        rhs=wo_sb[:, p, :],
                        start=True,
                        stop=True,
                    )
                    nc.any.tensor_copy(m_sb[:, p, :], mp[:, 0:C])
                w_sb = small_pool.tile([128, 2, C], F16, name="wsb", tag="wsb")
                for m in range(2):
                    wp = mm_pool.tile([128, HID], F32, name="wp", tag="mm")
                    for t in range(4):
                        nc.tensor.matmul(
                            wp[:, 0:C],
                            lhsT=wvT_sb[:, t, m * 128:(m + 1) * 128],
                            rhs=m_sb[:, t, :],
                            start=(t == 0),
                            stop=(t == 3),
                        )
                    nc.any.tensor_copy(w_sb[:, m, :], wp[:, 0:C])
                return w_sb

            def emit_y(b, w_sb):
                """y = x @ W per 128-row chunk; fp16 out, DMA from SBUF."""
                for k in range(32):
                    yp = mm_pool.tile([128, HID], F32, name="yp", tag="mm")
                    for t in range(2):
                        nc.tensor.matmul(
                            yp[:, 0:C],
                            lhsT=xt[b][:, t, k * 128:(k + 1) * 128],
                            rhs=w_sb[:, t, :],
                            start=(t == 0),
                            stop=(t == 1),
                        )
                    ysb = y_pool.tile([128, C], F16, name="ysb", tag="ysb")
                    nc.any.tensor_copy(ysb, yp[:, 0:C])
                    nc.sync.dma_start(
                        out=y_d[b, k * 128:(k + 1) * 128, :], in_=ysb
                    )

            # ---- schedule: G0 t2/sim0 | G1 (PE busy during softmax0) |
            #      M0 W0 Y0 | t2/sim1 M1 W1 Y1 ----
            g0 = emit_g(0)
            s0 = emit_t2_sim(0, g0)
            g1 = emit_g(1)
            e0 = emit_softmax(0, s0)
            w0 = emit_m_weff(0, e0)
            s1 = emit_t2_sim(1, g1)
            emit_y(0, w0)
            e1 = emit_softmax(1, s1)
            w1 = emit_m_weff(1, e1)
            emit_y(1, w1)
    return _split_multi_waits(nc)


def _get_nc():
    if "nc" not in _CACHE:
        _CACHE["nc"] = _build()
    return _CACHE["nc"]


def kernel(x, w_qkv, w_out, b_out, **kw):
    x = np.asarray(x, dtype=np.float32)
    w_qkv = np.asarray(w_qkv, dtype=np.float32)
    w_out = np.asarray(w_out, dtype=np.float32)
    b_out = np.asarray(b_out, dtype=np.float32)

    # fold q-scale (exact power of two) into w_q, quantize weights fp16
    wq = (w_qkv[:, :HID] * (64 ** -0.5)).astype(np.float16)
    wk = w_qkv[:, HID:2 * HID].astype(np.float16)
    wvT = np.ascontiguousarray(w_qkv[:, 2 * HID:].T).astype(np.float16)
    wo = w_out.astype(np.float16)
    # [c, j] -> [128, 2, j]: partition p holds channel t*128+p in slot t
    wq_r = np.ascontiguousarray(wq.reshape(2, 128, HID).transpose(1, 0, 2))
    wk_r = np.ascontiguousarray(wk.reshape(2, 128, HID).transpose(1, 0, 2))
    wvT_r = np.ascontiguousarray(wvT.reshape(4, 128, C).transpose(1, 0, 2))
    wo_r = np.ascontiguousarray(wo.reshape(4, 128, C).transpose(1, 0, 2))

    x4 = x.reshape(BATCH, D, C).astype(np.float16)
    in_maps = []
    for core in range(N_CORES):
        xb = x4[core * BPC:(core + 1) * BPC]  # [BPC, D, C]
        # d-major: partition p <- row k*128+p, free slot k
        x_dc = np.ascontiguousarray(
            xb.reshape(BPC, 32, 128, C).transpose(0, 2, 1, 3)
        ).reshape(BPC, 128, 32 * C)
        # c-major: partition p <- channel t*128+p, free slot t
        x_t = np.ascontiguousarray(
            xb.transpose(0, 2, 1).reshape(BPC, 2, 128, D).transpose(0, 2, 1, 3)
        ).reshape(BPC, 128, 2 * D)
        in_maps.append({
            "x_dc": x_dc, "xT": x_t,
            "wq_r": wq_r, "wk_r": wk_r, "wvT_r": wvT_r, "wo_r": wo_r,
        })

    nc = _get_nc()
    res = run_bass_kernel_spmd(nc, in_maps, core_ids=list(range(N_CORES)), **kw)
    y = np.concatenate([r["y"] for r in res.results], axis=0)  # [16, 4096, 256] f16
    y = y.astype(np.float32) + b_out
    return y.reshape(BATCH, 64, 64, C)


# revision 4
# speedup vs baseline: 2.8663x; 1.4363x over previous
"""Channel-attention Trainium2 Bass kernel, Gram-collapsed formulation.

Key identity: this is CHANNEL attention (the softmax mixes the 64 channels
of each head; every pixel is treated identically), so the whole module
collapses to a per-batch 256x256 effective channel-mixing matrix:

    G     = x^T x                     # [256,256] Gram, contracts d=4096
    sim_h = wq_h^T G wk_h             # [64,64] per head  (== (x wq)^T (x wk))
    attn_h = softmax(sim_h)           # denominator folded into wo rows
    M_h   = attn_h^T wo_h             # [64,256]
    W     = wv @ concat_h(M_h)        # [256,256] effective weight
    y     = x @ W (+ b_out)

Only G and y touch the [4096, 256] data; everything else is tiny (~5k of
the ~40k per-batch PE column-cycles). Per-batch PE cost ~40k vs ~164k for
the direct qkv formulation.

Distribution: data-parallel over batch - 8 cores x 2 batches each, weights
replicated, no collectives. Host supplies x twice in fp16 (d-major for G's
d-contraction, c-major for y's c-contraction), pre-folds the 1/8 q-scale
into w_q, and adds bias on the host. All matmuls fp16 x fp16 with fp32
PSUM accumulation (end-to-end rel-l2 ~2.4e-3 vs fp64 oracle).

Softmax denominators never touch e: attn = e/s is realized by scaling
wo's rows by r = 1/s (per-partition tensor_scalar) before the M matmul,
since M's contraction index (attn row i) is exactly wo's row index. e is
written straight out of the 4 grouped exp instructions as the fp16 lhsT
of the K=64 M matmuls.

DMA instruction count is minimized (17 per core: 1 weights + 2 xdc +
2 xT + 4 y per batch) because each DMA pays ~625ns of serialized HWDGE
issue overhead; transfers use >=4KB contiguous per-partition runs to
stay at full 360GB/s.

The Y phase keeps W stationary and streams xT as the moving operand, so
PSUM drains are 16 [128,512] copies instead of 32 [128,256] ones, and y
leaves the core as yT [256, 4096] fp16 (host transposes back).
"""

import numpy as np

import concourse.bass as bass
import concourse.mybir as mybir
from concourse.bass_utils import run_bass_kernel_spmd
from concourse.tile import TileContext


def _split_multi_waits(nc, limit=1):
    """Post-pass: the walrus build in this container rejects instructions
    carrying more than `limit` sync-waits ("Too many sync wait commands" in
    setupSyncWait). Tile attaches up to 3. Hoist the extras onto same-engine
    NoOp instructions inserted immediately before the owner — the engine
    sequencer executes them in order, so the ordering semantics are
    identical."""
    drain_engines = [
        mybir.EngineType.PE,
        mybir.EngineType.DVE,
        mybir.EngineType.Activation,
        mybir.EngineType.Pool,
        mybir.EngineType.SP,
    ]
    n_split = 0
    for f in nc.m.functions:
        for blk in f.blocks:
            il = blk.instructions
            i = 0
            while i < len(il):
                inst = il[i]
                si = inst.sync_info
                waits = list(si.on_wait) if si is not None else []
                if len(waits) > limit:
                    si.on_wait = waits[:limit]
                    is_drain = type(inst).__name__ == "InstDrain"
                    for k, w in enumerate(waits[limit:]):
                        nop = mybir.InstNoOp(
                            name=f"I-waitsplit-{n_split}", ins=[], outs=[]
                        )
                        n_split += 1
                        nop.engine = (
                            drain_engines[k % len(drain_engines)]
                            if is_drain else inst.engine
                        )
                        nop.sync_info = mybir.SyncInfo(on_wait=[w], on_update=[])
                        il.insert(i, nop)
                        i += 1
                i += 1
    return nc


N_CORES = 8
BATCH = 16
BPC = BATCH // N_CORES  # batches per core
D = 4096  # spatial (64*64)
C = 256   # channels
HID = 512
HEADS = 8

F32 = mybir.dt.float32
F16 = mybir.dt.float16

# offsets into the packed weight tile w_all [128, 4096] (fp16)
WK_OFF = 0          # wk  [128, 2, 512]
WQ_OFF = 1024       # wq' [128, 2, 512]  (q-scale folded)
WVT_OFF = 2048      # wvT [128, 4, 256]
WO_OFF = 3072       # wo  [128, 4, 256]

_CACHE = {}


def _build():
    nc = bass.Bass()
    # x twice: d-major (partition = d%128) for G, c-major for Y
    xdc_d = nc.declare_dram_parameter("x_dc", [BPC, 128, 32 * C], F16, isOutput=False)
    xt_d = nc.declare_dram_parameter("xT", [BPC, 128, 2 * D], F16, isOutput=False)
    w_d = nc.declare_dram_parameter("w_all", [128, 4096], F16, isOutput=False)
    y_d = nc.declare_dram_parameter("y", [BPC, 128, 2 * D], F16, isOutput=True)

    with TileContext(nc) as tc:
        with (
            tc.tile_pool(name="consts", bufs=1) as consts,
            tc.tile_pool(name="xdc", bufs=2) as xdc_pool,
            tc.tile_pool(name="xt", bufs=2) as xt_pool,
            tc.tile_pool(name="small", bufs=2) as small_pool,
            tc.tile_pool(name="e64", bufs=2) as e_pool,
            tc.tile_pool(name="stat", bufs=6) as stat_pool,
            tc.tile_pool(name="ysb", bufs=2) as y_pool,
            tc.tile_pool(name="mm", bufs=6, space="PSUM") as mm_pool,
            tc.tile_pool(name="simp", bufs=2, space="PSUM") as sim_pool,
        ):
            w_all = consts.tile([128, 4096], F16, name="w_all")

            def wk(j):  # [128, 512] chunk j of wk (c-chunk)
                return w_all[:, WK_OFF + j * HID:WK_OFF + (j + 1) * HID]

            def wq(j, h=None):
                lo = WQ_OFF + j * HID
                if h is None:
                    return w_all[:, lo:lo + HID]
                return w_all[:, lo + h * 64:lo + (h + 1) * 64]

            def wvt(t, m):  # [128, 128]: hid-chunk t, c-half m
                lo = WVT_OFF + t * C + m * 128
                return w_all[:, lo:lo + 128]

            def wo(p):  # [128, 256] rows of head pair p
                lo = WO_OFF + p * C
                return w_all[:, lo:lo + C]

            # ---- DMAs: w -> xdc(b0) -> xdc(b1) -> xT(b0) -> xT(b1) ----
            nc.sync.dma_start(out=w_all, in_=w_d[:, :])
            xdc, xt = [], []
            for b in range(BPC):
                x_t = xdc_pool.tile([128, 32, C], F16, name=f"xdc{b}", tag="xdc")
                xdc.append(x_t)
                t_t = xt_pool.tile([128, 2, D], F16, name=f"xt{b}", tag="xt")
                xt.append(t_t)
            for b in range(BPC):
                for ci in range(2):  # 16 d-chunks per DMA -> [128, 8KB]
                    nc.sync.dma_start(
                        out=xdc[b][:, ci * 16:(ci + 1) * 16, :],
                        in_=xdc_d[b, :, ci * 16 * C:(ci + 1) * 16 * C],
                    )
            for b in range(BPC):
                for t in range(2):  # [128, 8KB] per c-half
                    nc.sync.dma_start(
                        out=xt[b][:, t, :],
                        in_=xt_d[b, :, t * D:(t + 1) * D],
                    )

            def emit_g(b):
                """G = x^T x accumulated over 32 d-chunks; -> Gsb fp16."""
                gps = [
                    mm_pool.tile([128, HID], F32, name=f"gps{m}", tag="mm")
                    for m in range(2)
                ]
                for k in range(32):
                    for m in range(2):
                        nc.tensor.matmul(
                            gps[m][:, 0:C],
                            lhsT=xdc[b][:, k, m * 128:(m + 1) * 128],
                            rhs=xdc[b][:, k, :],
                            start=(k == 0),
                            stop=(k == 31),
                        )
                g_sb = small_pool.tile([128, 2, C], F16, name="gsb", tag="gsb")
                for m in range(2):
                    nc.any.tensor_copy(g_sb[:, m, :], gps[m][:, 0:C])
                return g_sb

            def emit_t2_sim(b, g_sb):
                """t2 = G @ wk -> fp16; sim_h = wq_h^T t2_h -> PSUM.

                G is exactly symmetric (both halves accumulate the same
                products in the same order), so Gsb tile j doubles as the
                [c2-chunk j, c1] stationary operand."""
                t2_sb = small_pool.tile([128, 2, HID], F16, name="t2sb", tag="t2")
                for m in range(2):
                    t2p = mm_pool.tile([128, HID], F32, name="t2p", tag="mm")
                    for j in range(2):
                        nc.tensor.matmul(
                            t2p,
                            lhsT=g_sb[:, j, m * 128:(m + 1) * 128],
                            rhs=wk(j),
                            start=(j == 0),
                            stop=(j == 1),
                        )
                    nc.any.tensor_copy(t2_sb[:, m, :], t2p)
                # sim packing: head h=(2p+par) -> rows par*64:+64, cols
                # p*64:+64 of sim_all [128, 256]
                sim_all = sim_pool.tile([128, HID], F32, name="sim", tag="simp")
                nc.vector.memset(sim_all[:, 0:C], 0.0)
                for h in range(HEADS):
                    par, p = h % 2, h // 2
                    rows = slice(par * 64, par * 64 + 64)
                    for j in range(2):
                        nc.tensor.matmul(
                            sim_all[rows, p * 64:(p + 1) * 64],
                            lhsT=wq(j, h),
                            rhs=t2_sb[:, j, h * 64:(h + 1) * 64],
                            start=False,
                            stop=(j == 1),
                            skip_group_check=True,
                        )
                return sim_all

            def emit_softmax_m_weff(b, sim_all):
                """exp (grouped, max-subtracted) -> e64 fp16; 1/s folded
                into wo rows; M via K=64 matmuls; W = wv @ M -> Wsb fp16."""
                m_t = stat_pool.tile([128, 4], F32, name="m_t", tag="stat")
                s_t = stat_pool.tile([128, 4], F32, name="s_t", tag="stat")
                r_t = stat_pool.tile([128, 4], F32, name="r_t", tag="stat")
                e64 = e_pool.tile([128, 4, 64], F16, name="e64", tag="e64")
                # neg-max per (partition, head-block): [64,4,64] -> [64,4]
                for par in range(2):
                    rows = slice(par * 64, par * 64 + 64)
                    nc.vector.reduce_max(
                        out=m_t[rows, 0:4],
                        in_=sim_all[rows, 0:C].rearrange("p (g j) -> p g j", g=4),
                        axis=mybir.AxisListType.X,
                        negate=True,
                    )
                for p in range(4):
                    nc.scalar.activation(
                        out=e64[:, p, :],
                        in_=sim_all[:, p * 64:(p + 1) * 64],
                        func=mybir.ActivationFunctionType.Exp,
                        bias=m_t[:, p:p + 1],
                        scale=1.0,
                        accum_out=s_t[:, p:p + 1],
                    )
                nc.vector.reciprocal(r_t, s_t)
                m_sb = small_pool.tile([128, 4, C], F16, name="msb", tag="msb")
                for p in range(4):
                    wop = stat_pool.tile([128, C], F16, name=f"wop{p}", tag="wop")
                    nc.vector.tensor_scalar_mul(wop, wo(p), r_t[:, p:p + 1])
                    mp = mm_pool.tile([128, HID], F32, name="mp", tag="mm")
                    for par in range(2):
                        rows = slice(par * 64, par * 64 + 64)
                        nc.tensor.matmul(
                            mp[rows, 0:C],
                            lhsT=e64[rows, p, :],
                            rhs=wop[rows, :],
                            start=True,
                            stop=True,
                        )
                    nc.any.tensor_copy(m_sb[:, p, :], mp[:, 0:C])
                w_sb = small_pool.tile([128, 2, C], F16, name="wsb", tag="wsb")
                for m in range(2):
                    wp = mm_pool.tile([128, HID], F32, name="wp", tag="mm")
                    for t in range(4):
                        nc.tensor.matmul(
                            wp[:, 0:C],
                            lhsT=wvt(t, m),
                            rhs=m_sb[:, t, :],
                            start=(t == 0),
                            stop=(t == 3),
                        )
                    nc.any.tensor_copy(w_sb[:, m, :], wp[:, 0:C])
                return w_sb

            def emit_y(b, w_sb):
                """yT = W^T x^T: W halves stationary, xT moving."""
                yt_sb = y_pool.tile([128, 2, D], F16, name="ysb", tag="ysb")
                for m in range(2):
                    for d5 in range(8):
                        yp = mm_pool.tile([128, HID], F32, name="yp", tag="mm")
                        for t in range(2):
                            nc.tensor.matmul(
                                yp,
                                lhsT=w_sb[:, t, m * 128:(m + 1) * 128],
                                rhs=xt[b][:, t, d5 * 512:(d5 + 1) * 512],
                                start=(t == 0),
                                stop=(t == 1),
                            )
                        nc.any.tensor_copy(
                            yt_sb[:, m, d5 * 512:(d5 + 1) * 512], yp
                        )
                    for half in range(2):
                        lo = half * 2048
                        nc.sync.dma_start(
                            out=y_d[b, :, m * D + lo:m * D + lo + 2048],
                            in_=yt_sb[:, m, lo:lo + 2048],
                        )

            # ---- schedule: G0 t2/sim0 | G1 (PE busy during softmax0) |
            #      M0 W0 Y0 | t2/sim1 M1 W1 Y1 ----
            g0 = emit_g(0)
            s0 = emit_t2_sim(0, g0)
            g1 = emit_g(1)
            w0 = emit_softmax_m_weff(0, s0)
            s1 = emit_t2_sim(1, g1)
            emit_y(0, w0)
            w1 = emit_softmax_m_weff(1, s1)
            emit_y(1, w1)
    return _split_multi_waits(nc)


def _get_nc():
    if "nc" not in _CACHE:
        _CACHE["nc"] = _build()
    return _CACHE["nc"]


def kernel(x, w_qkv, w_out, b_out, **kw):
    x = np.asarray(x, dtype=np.float32)
    w_qkv = np.asarray(w_qkv, dtype=np.float32)
    w_out = np.asarray(w_out, dtype=np.float32)
    b_out = np.asarray(b_out, dtype=np.float32)

    # fold q-scale (exact power of two) into w_q; pack weights into one
    # [128, 4096] fp16 tile: [wk | wq' | wvT | wo], each c/hid-chunked so
    # partition p holds row t*128+p of the logical matrix in slot t
    wq = (w_qkv[:, :HID] * (64 ** -0.5)).astype(np.float16)
    wk = w_qkv[:, HID:2 * HID].astype(np.float16)
    wvT = np.ascontiguousarray(w_qkv[:, 2 * HID:].T).astype(np.float16)
    wo = w_out.astype(np.float16)
    w_all = np.concatenate([
        wk.reshape(2, 128, HID).transpose(1, 0, 2).reshape(128, 2 * HID),
        wq.reshape(2, 128, HID).transpose(1, 0, 2).reshape(128, 2 * HID),
        wvT.reshape(4, 128, C).transpose(1, 0, 2).reshape(128, 4 * C),
        wo.reshape(4, 128, C).transpose(1, 0, 2).reshape(128, 4 * C),
    ], axis=1)
    w_all = np.ascontiguousarray(w_all)

    x4 = x.reshape(BATCH, D, C).astype(np.float16)
    in_maps = []
    for core in range(N_CORES):
        xb = x4[core * BPC:(core + 1) * BPC]  # [BPC, D, C]
        # d-major: partition p <- row k*128+p, free slot k
        x_dc = np.ascontiguousarray(
            xb.reshape(BPC, 32, 128, C).transpose(0, 2, 1, 3)
        ).reshape(BPC, 128, 32 * C)
        # c-major: partition p <- channel t*128+p, free slot t
        x_t = np.ascontiguousarray(
            xb.transpose(0, 2, 1).reshape(BPC, 2, 128, D).transpose(0, 2, 1, 3)
        ).reshape(BPC, 128, 2 * D)
        in_maps.append({"x_dc": x_dc, "xT": x_t, "w_all": w_all})

    nc = _get_nc()
    res = run_bass_kernel_spmd(nc, in_maps, core_ids=list(range(N_CORES)), **kw)
    # y arrives as yT [BPC, 128, 2, D]: channel t*128+p, pixel d
    yt = np.stack([r["y"] for r in res.results])  # [cores, BPC, 128, 2*D]
    yt = yt.reshape(BATCH, 128, 2, D).transpose(0, 2, 1, 3).reshape(BATCH, C, D)
    y = yt.transpose(0, 2, 1).astype(np.float32) + b_out
    return y.reshape(BATCH, 64, 64, C)


# revision 5
# speedup vs baseline: 3.4534x; 1.2048x over previous
"""Channel-attention Trainium2 Bass kernel, Gram-collapsed + fp8 DoubleRow.

Key identity: this is CHANNEL attention (the softmax mixes the 64 channels
of each head; every pixel is treated identically), so the whole module
collapses to a per-batch 256x256 effective channel-mixing matrix:

    G     = x^T x                     # [256,256] Gram, contracts d=4096
    sim_h = wq_h^T G wk_h             # [64,64] per head  (== (x wq)^T (x wk))
    attn_h = softmax(sim_h)           # denominator folded into wo rows
    M_h   = attn_h^T wo_h             # [64,256]
    W     = wv @ concat_h(M_h)        # [256,256] effective weight
    y     = x @ W (+ b_out)

Only G and y touch the [4096, 256] data; both run as fp8e4m3 DoubleRow
matmuls (0.5 cyc/row, 2 K-tiles per instruction) on hi+lo residual pairs:
a @ b ~= ah@bh + al@bh + ah@bl, where the lo tensors carry the fp8
quantization residual of the hi ones. That keeps fp16-grade accuracy
(end-to-end rel-l2 ~3.4e-3 vs the fp64 oracle) at fp8 speed and the same
DMA bytes as fp16. Scales are powers of two: x_dc*2 (so Gsb=4G stays
under fp16 max), xT*16, W*256; the q-scale/8, G/4 land in w_q host-side
and the 4096x on y divides out on the host.

Softmax denominators never touch e: attn = e/s is realized by scaling
wo's rows by r = 1/s (per-partition tensor_scalar) before the M matmul,
since M's contraction index (attn row i) is exactly wo's row index.

Distribution: data-parallel over batch - 8 cores x 2 batches each, weights
replicated, no collectives. Per-core DMA is the roofline (~13MB at
360GB/s ~= 38us): x twice (d-major for G, c-major for y, 2MB/batch each),
y out fp16 (2MB/batch), weights 1MB. DMA instruction count stays small
(~29/core, ~625ns serialized issue each) and every transfer keeps >=2KB
contiguous per-partition runs for full bandwidth. PE work is ~32k
column-cycles/batch (~27us/core), hidden under the DMA stream.
"""

import numpy as np
import ml_dtypes

import concourse.bass as bass
import concourse.mybir as mybir
from concourse.bass_utils import run_bass_kernel_spmd
from concourse.tile import TileContext

DR = mybir.MatmulPerfMode.DoubleRow


def _split_multi_waits(nc, limit=1):
    """Post-pass: the walrus build in this container rejects instructions
    carrying more than `limit` sync-waits ("Too many sync wait commands" in
    setupSyncWait). Tile attaches up to 3. Hoist the extras onto same-engine
    NoOp instructions inserted immediately before the owner — the engine
    sequencer executes them in order, so the ordering semantics are
    identical."""
    drain_engines = [
        mybir.EngineType.PE,
        mybir.EngineType.DVE,
        mybir.EngineType.Activation,
        mybir.EngineType.Pool,
        mybir.EngineType.SP,
    ]
    n_split = 0
    for f in nc.m.functions:
        for blk in f.blocks:
            il = blk.instructions
            i = 0
            while i < len(il):
                inst = il[i]
                si = inst.sync_info
                waits = list(si.on_wait) if si is not None else []
                if len(waits) > limit:
                    si.on_wait = waits[:limit]
                    is_drain = type(inst).__name__ == "InstDrain"
                    for k, w in enumerate(waits[limit:]):
                        nop = mybir.InstNoOp(
                            name=f"I-waitsplit-{n_split}", ins=[], outs=[]
                        )
                        n_split += 1
                        nop.engine = (
                            drain_engines[k % len(drain_engines)]
                            if is_drain else inst.engine
                        )
                        nop.sync_info = mybir.SyncInfo(on_wait=[w], on_update=[])
                        il.insert(i, nop)
                        i += 1
                i += 1
    return nc


N_CORES = 8
BATCH = 16
BPC = BATCH // N_CORES  # batches per core
D = 4096  # spatial (64*64)
C = 256   # channels
HID = 512
HEADS = 8

F32 = mybir.dt.float32
F16 = mybir.dt.float16
F8 = mybir.dt.float8e4
E4NP = ml_dtypes.float8_e4m3

# offsets into the packed weight tile w_all [128, 4096] (fp16)
WK_OFF = 0          # wk  [128, 2, 512]
WQ_OFF = 1024       # wq' [128, 2, 512]  (q-scale/8 and Gram-scale/4 folded)
WVT_OFF = 2048      # wvT [128, 4, 256]
WO_OFF = 3072       # wo  [128, 4, 256]

_CACHE = {}


def _build():
    nc = bass.Bass()
    # x twice: d-major (partition = d%128) for G, c-major for Y; each as an
    # fp8 hi/lo residual pair (same bytes as fp16)
    xdh_d = nc.declare_dram_parameter("xdc_hi", [BPC, 128, 32 * C], F8, isOutput=False)
    xdl_d = nc.declare_dram_parameter("xdc_lo", [BPC, 128, 32 * C], F8, isOutput=False)
    xth_d = nc.declare_dram_parameter("xT_hi", [BPC, 128, 2 * D], F8, isOutput=False)
    xtl_d = nc.declare_dram_parameter("xT_lo", [BPC, 128, 2 * D], F8, isOutput=False)
    w_d = nc.declare_dram_parameter("w_all", [128, 4096], F16, isOutput=False)
    y_d = nc.declare_dram_parameter("y", [BPC, 128, 2 * D], F16, isOutput=True)

    with TileContext(nc) as tc:
        with (
            tc.tile_pool(name="consts", bufs=1) as consts,
            tc.tile_pool(name="xdc", bufs=2) as xdc_pool,
            tc.tile_pool(name="xt", bufs=2) as xt_pool,
            tc.tile_pool(name="small", bufs=2) as small_pool,
            tc.tile_pool(name="e64", bufs=2) as e_pool,
            tc.tile_pool(name="stat", bufs=6) as stat_pool,
            tc.tile_pool(name="ysb", bufs=2) as y_pool,
            tc.tile_pool(name="mm", bufs=6, space="PSUM") as mm_pool,
            tc.tile_pool(name="simp", bufs=2, space="PSUM") as sim_pool,
        ):
            w_all = consts.tile([128, 4096], F16, name="w_all")

            def wk(j):  # [128, 512] c-chunk j
                return w_all[:, WK_OFF + j * HID:WK_OFF + (j + 1) * HID]

            def wq(j, h):  # [128, 64] c-chunk j, head h
                lo = WQ_OFF + j * HID + h * 64
                return w_all[:, lo:lo + 64]

            def wvt(t, m):  # [128, 128]: hid-chunk t, c-half m
                lo = WVT_OFF + t * C + m * 128
                return w_all[:, lo:lo + 128]

            def wo(p):  # [128, 256] rows of head pair p
                lo = WO_OFF + p * C
                return w_all[:, lo:lo + C]

            # ---- x/w tiles + DMA stream (order = issue order) ----
            xdh, xdl, xth, xtl = [], [], [], []
            for b in range(BPC):
                xdh.append(xdc_pool.tile([128, 32, C], F8, name=f"xdh{b}", tag="xdh"))
                xdl.append(xdc_pool.tile([128, 32, C], F8, name=f"xdl{b}", tag="xdl"))
                xth.append(xt_pool.tile([128, 2, D], F8, name=f"xth{b}", tag="xth"))
                xtl.append(xt_pool.tile([128, 2, D], F8, name=f"xtl{b}", tag="xtl"))

            def dma_xdc(b, half):
                ks = slice(half * 16, (half + 1) * 16)
                el = slice(half * 16 * C, (half + 1) * 16 * C)
                nc.sync.dma_start(out=xdh[b][:, ks, :], in_=xdh_d[b, :, el])
                nc.sync.dma_start(out=xdl[b][:, ks, :], in_=xdl_d[b, :, el])

            dma_xdc(0, 0)
            dma_xdc(0, 1)
            nc.sync.dma_start(out=w_all[:, 0:2048], in_=w_d[:, 0:2048])
            dma_xdc(1, 0)
            dma_xdc(1, 1)
            nc.sync.dma_start(out=w_all[:, 2048:4096], in_=w_d[:, 2048:4096])
            for b in range(BPC):
                nc.sync.dma_start(out=xth[b], in_=xth_d[b, :, :])
                nc.sync.dma_start(out=xtl[b], in_=xtl_d[b, :, :])

            def emit_g(b):
                """G = (xh+xl)^T(xh+xl) (3-term) via fp8 DoubleRow over
                d-chunk pairs; PSUM = 4G -> Gsb fp16. Emitted half-by-half
                so the first half's matmuls start under the second's DMA."""
                gps = [
                    mm_pool.tile([128, HID], F32, name=f"gps{m}", tag="mm")
                    for m in range(2)
                ]
                for half in range(2):
                    for ab, (lt, rt) in enumerate(
                        ((xdh, xdh), (xdl, xdh), (xdh, xdl))
                    ):
                        for pp in range(8):
                            p2 = half * 16 + pp * 2
                            ks = slice(p2, p2 + 2)
                            for m in range(2):
                                nc.tensor.matmul(
                                    gps[m][:, 0:C],
                                    lhsT=lt[b][:, ks, m * 128:(m + 1) * 128],
                                    rhs=rt[b][:, ks, :],
                                    start=(half == 0 and ab == 0 and pp == 0),
                                    stop=(half == 1 and ab == 2 and pp == 7),
                                    perf_mode=DR,
                                )
                g_sb = small_pool.tile([128, 2, C], F16, name="gsb", tag="gsb")
                for m in range(2):
                    nc.any.tensor_copy(g_sb[:, m, :], gps[m][:, 0:C])
                return g_sb

            def emit_t2_sim(b, g_sb):
                """t2 = G @ wk -> fp16; sim_h = wq_h^T t2_h -> PSUM.

                G is exactly symmetric (both halves accumulate the same
                products in the same order), so Gsb tile j doubles as the
                [c2-chunk j, c1] stationary operand."""
                t2_sb = small_pool.tile([128, 2, HID], F16, name="t2sb", tag="t2")
                for m in range(2):
                    t2p = mm_pool.tile([128, HID], F32, name="t2p", tag="mm")
                    for j in range(2):
                        nc.tensor.matmul(
                            t2p,
                            lhsT=g_sb[:, j, m * 128:(m + 1) * 128],
                            rhs=wk(j),
                            start=(j == 0),
                            stop=(j == 1),
                        )
                    nc.any.tensor_copy(t2_sb[:, m, :], t2p)
                # sim packing: head h=(2p+par) -> rows par*64:+64, cols
                # p*64:+64 of sim_all [128, 256]
                sim_all = sim_pool.tile([128, HID], F32, name="sim", tag="simp")
                nc.vector.memset(sim_all[:, 0:C], 0.0)
                for h in range(HEADS):
                    par, p = h % 2, h // 2
                    rows = slice(par * 64, par * 64 + 64)
                    for j in range(2):
                        nc.tensor.matmul(
                            sim_all[rows, p * 64:(p + 1) * 64],
                            lhsT=wq(j, h),
                            rhs=t2_sb[:, j, h * 64:(h + 1) * 64],
                            start=False,
                            stop=(j == 1),
                            skip_group_check=True,
                        )
                return sim_all

            def emit_softmax_m_weff(b, sim_all):
                """exp (grouped, max-subtracted) -> e64 fp16; 1/s folded
                into wo rows; M via K=64 matmuls; W = wv @ M -> fp8 hi/lo
                pair at scale 256."""
                m_t = stat_pool.tile([128, 4], F32, name="m_t", tag="stat")
                s_t = stat_pool.tile([128, 4], F32, name="s_t", tag="stat")
                r_t = stat_pool.tile([128, 4], F32, name="r_t", tag="stat")
                e64 = e_pool.tile([128, 4, 64], F16, name="e64", tag="e64")
                # neg-max per (partition, head-block): [64,4,64] -> [64,4]
                for par in range(2):
                    rows = slice(par * 64, par * 64 + 64)
                    nc.vector.reduce_max(
                        out=m_t[rows, 0:4],
                        in_=sim_all[rows, 0:C].rearrange("p (g j) -> p g j", g=4),
                        axis=mybir.AxisListType.X,
                        negate=True,
                    )
                for p in range(4):
                    nc.scalar.activation(
                        out=e64[:, p, :],
                        in_=sim_all[:, p * 64:(p + 1) * 64],
                        func=mybir.ActivationFunctionType.Exp,
                        bias=m_t[:, p:p + 1],
                        scale=1.0,
                        accum_out=s_t[:, p:p + 1],
                    )
                nc.vector.reciprocal(r_t, s_t)
                m_sb = small_pool.tile([128, 4, C], F16, name="msb", tag="msb")
                for p in range(4):
                    wop = stat_pool.tile([128, C], F16, name=f"wop{p}", tag="wop")
                    nc.vector.tensor_scalar_mul(wop, wo(p), r_t[:, p:p + 1])
                    mp = mm_pool.tile([128, HID], F32, name="mp", tag="mm")
                    for par in range(2):
                        rows = slice(par * 64, par * 64 + 64)
                        nc.tensor.matmul(
                            mp[rows, 0:C],
                            lhsT=e64[rows, p, :],
                            rhs=wop[rows, :],
                            start=True,
                            stop=True,
                        )
                    nc.any.tensor_copy(m_sb[:, p, :], mp[:, 0:C])
                ws16 = small_pool.tile([128, 2, C], F16, name="ws16", tag="ws16")
                w_hi = small_pool.tile([128, 2, C], F8, name="whi", tag="whi")
                w_lo = small_pool.tile([128, 2, C], F8, name="wlo", tag="wlo")
                for m in range(2):
                    wp = mm_pool.tile([128, HID], F32, name="wp", tag="mm")
                    for t in range(4):
                        nc.tensor.matmul(
                            wp[:, 0:C],
                            lhsT=wvt(t, m),
                            rhs=m_sb[:, t, :],
                            start=(t == 0),
                            stop=(t == 3),
                        )
                    # W * 256 as fp8 hi + residual lo
                    nc.vector.tensor_scalar_mul(ws16[:, m, :], wp[:, 0:C], 256.0)
                    nc.any.tensor_copy(w_hi[:, m, :], ws16[:, m, :])
                    nc.vector.tensor_sub(w_lo[:, m, :], ws16[:, m, :], w_hi[:, m, :])
                return w_hi, w_lo

            def emit_y(b, w_hi, w_lo):
                """yT = W^T x^T: fp8 DoubleRow, W halves stationary, xT
                moving; PSUM = 4096*y -> fp16 (host divides)."""
                yt_sb = y_pool.tile([128, 2, D], F16, name="ysb", tag="ysb")
                for d4 in range(4):
                    for m in range(2):
                        for dd in range(2):
                            d5 = d4 * 2 + dd
                            cols = slice(d5 * 512, (d5 + 1) * 512)
                            yp = mm_pool.tile([128, HID], F32, name="yp", tag="mm")
                            for ti, (lt, rt) in enumerate(
                                ((w_hi, xth[b]), (w_lo, xth[b]), (w_hi, xtl[b]))
                            ):
                                nc.tensor.matmul(
                                    yp,
                                    lhsT=lt[:, :, m * 128:(m + 1) * 128],
                                    rhs=rt[:, :, cols],
                                    start=(ti == 0),
                                    stop=(ti == 2),
                                    perf_mode=DR,
                                )
                            nc.any.tensor_copy(yt_sb[:, m, cols], yp)
                        lo = d4 * 1024
                        nc.sync.dma_start(
                            out=y_d[b, :, m * D + lo:m * D + lo + 1024],
                            in_=yt_sb[:, m, lo:lo + 1024],
                        )

            # ---- schedule: G0 t2/sim0 | G1 (PE busy during softmax0) |
            #      M0 W0 Y0 | t2/sim1 M1 W1 Y1 ----
            g0 = emit_g(0)
            s0 = emit_t2_sim(0, g0)
            g1 = emit_g(1)
            wh0, wl0 = emit_softmax_m_weff(0, s0)
            s1 = emit_t2_sim(1, g1)
            emit_y(0, wh0, wl0)
            wh1, wl1 = emit_softmax_m_weff(1, s1)
            emit_y(1, wh1, wl1)
    return _split_multi_waits(nc)


def _get_nc():
    if "nc" not in _CACHE:
        _CACHE["nc"] = _build()
    return _CACHE["nc"]


def _hilo(x, scale):
    """fp8e4m3 hi + residual lo of x*scale (f32 in, ml_dtypes out)."""
    xs = (x * scale).astype(np.float32)
    hi = xs.astype(E4NP)
    lo = (xs - hi.astype(np.float32)).astype(E4NP)
    return hi, lo


def kernel(x, w_qkv, w_out, b_out, **kw):
    x = np.asarray(x, dtype=np.float32)
    w_qkv = np.asarray(w_qkv, dtype=np.float32)
    w_out = np.asarray(w_out, dtype=np.float32)
    b_out = np.asarray(b_out, dtype=np.float32)

    # fold q-scale/8 and Gram-scale/4 into w_q; pack weights into one
    # [128, 4096] fp16 tile: [wk | wq' | wvT | wo], each c/hid-chunked so
    # partition p holds row t*128+p of the logical matrix in slot t
    wq = (w_qkv[:, :HID] * (64 ** -0.5) * 0.25).astype(np.float16)
    wk = w_qkv[:, HID:2 * HID].astype(np.float16)
    wvT = np.ascontiguousarray(w_qkv[:, 2 * HID:].T).astype(np.float16)
    wo = w_out.astype(np.float16)
    w_all = np.concatenate([
        wk.reshape(2, 128, HID).transpose(1, 0, 2).reshape(128, 2 * HID),
        wq.reshape(2, 128, HID).transpose(1, 0, 2).reshape(128, 2 * HID),
        wvT.reshape(4, 128, C).transpose(1, 0, 2).reshape(128, 4 * C),
        wo.reshape(4, 128, C).transpose(1, 0, 2).reshape(128, 4 * C),
    ], axis=1)
    w_all = np.ascontiguousarray(w_all)

    x4 = x.reshape(BATCH, D, C)
    in_maps = []
    for core in range(N_CORES):
        xb = x4[core * BPC:(core + 1) * BPC]  # [BPC, D, C] f32
        # d-major: partition p <- row k*128+p, free slot k; scale 2
        x_dc = np.ascontiguousarray(
            xb.reshape(BPC, 32, 128, C).transpose(0, 2, 1, 3)
        ).reshape(BPC, 128, 32 * C)
        xdh, xdl = _hilo(x_dc, 2.0)
        # c-major: partition p <- channel t*128+p, free slot t; scale 16
        x_t = np.ascontiguousarray(
            xb.transpose(0, 2, 1).reshape(BPC, 2, 128, D).transpose(0, 2, 1, 3)
        ).reshape(BPC, 128, 2 * D)
        xth, xtl = _hilo(x_t, 16.0)
        in_maps.append({
            "xdc_hi": xdh, "xdc_lo": xdl, "xT_hi": xth, "xT_lo": xtl,
            "w_all": w_all,
        })

    nc = _get_nc()
    res = run_bass_kernel_spmd(nc, in_maps, core_ids=list(range(N_CORES)), **kw)
    # y arrives as yT*4096 [BPC, 128, 2, D]: channel t*128+p, pixel d
    yt = np.stack([r["y"] for r in res.results])  # [cores, BPC, 128, 2*D]
    yt = yt.reshape(BATCH, 128, 2, D).transpose(0, 2, 1, 3).reshape(BATCH, C, D)
    y = yt.transpose(0, 2, 1).astype(np.float32) * (2.0 ** -12) + b_out
    return y.reshape(BATCH, 64, 64, C)


# revision 14
# speedup vs baseline: 3.5451x; 1.0266x over previous
"""Channel-attention Trainium2 Bass kernel, Gram-collapsed + fp8 DoubleRow.

Key identity: this is CHANNEL attention (the softmax mixes the 64 channels
of each head; every pixel is treated identically), so the whole module
collapses to a per-batch 256x256 effective channel-mixing matrix:

    G     = x^T x                     # [256,256] Gram, contracts d=4096
    sim_h = wq_h^T G wk_h             # [64,64] per head  (== (x wq)^T (x wk))
    attn_h = softmax(sim_h)           # denominator folded into wo rows
    M_h   = attn_h^T wo_h             # [64,256]
    W     = wv @ concat_h(M_h)        # [256,256] effective weight
    y     = x @ W (+ b_out)

Only G and y touch the [4096, 256] data; both run as fp8e4m3 DoubleRow
matmuls (0.5 cyc/row, 2 K-tiles per instruction) on hi+lo residual pairs:
a @ b ~= ah@bh + al@bh + ah@bl, where the lo tensors carry the fp8
quantization residual of the hi ones. That keeps fp16-grade accuracy
(end-to-end rel-l2 ~3.4e-3 vs the fp64 oracle) at fp8 speed and the same
DMA bytes as fp16. Scales are powers of two: x_dc*2 (so Gsb=4G stays
under fp16 max), xT*16, W*256; the q-scale/8, G/4 land in w_q host-side
and the 4096x on y divides out on the host.

Softmax denominators never touch e: attn = e/s is realized by scaling
wo's rows by r = 1/s (per-partition tensor_scalar) before the M matmul,
since M's contraction index (attn row i) is exactly wo's row index.

Distribution: data-parallel over batch - 8 cores x 2 batches each, weights
replicated, no collectives. Per-core DMA is the roofline (~13MB at
360GB/s ~= 38us): x twice (d-major for G, c-major for y, 2MB/batch each),
y out fp16 (2MB/batch), weights 1MB. DMA instruction count stays small
(~29/core, ~625ns serialized issue each) and every transfer keeps >=2KB
contiguous per-partition runs for full bandwidth. PE work is ~32k
column-cycles/batch (~27us/core), hidden under the DMA stream.
"""

import numpy as np
import ml_dtypes

import concourse.bass as bass
import concourse.mybir as mybir
from concourse.bass_utils import run_bass_kernel_spmd
from concourse.tile import TileContext

DR = mybir.MatmulPerfMode.DoubleRow


def _split_multi_waits(nc, limit=1):
    """Post-pass: the walrus build in this container rejects instructions
    carrying more than `limit` sync-waits ("Too many sync wait commands" in
    setupSyncWait). Tile attaches up to 3. Hoist the extras onto same-engine
    NoOp instructions inserted immediately before the owner — the engine
    sequencer executes them in order, so the ordering semantics are
    identical."""
    drain_engines = [
        mybir.EngineType.PE,
        mybir.EngineType.DVE,
        mybir.EngineType.Activation,
        mybir.EngineType.Pool,
        mybir.EngineType.SP,
    ]
    n_split = 0
    for f in nc.m.functions:
        for blk in f.blocks:
            il = blk.instructions
            i = 0
            while i < len(il):
                inst = il[i]
                si = inst.sync_info
                waits = list(si.on_wait) if si is not None else []
                if len(waits) > limit:
                    si.on_wait = waits[:limit]
                    is_drain = type(inst).__name__ == "InstDrain"
                    for k, w in enumerate(waits[limit:]):
                        nop = mybir.InstNoOp(
                            name=f"I-waitsplit-{n_split}", ins=[], outs=[]
                        )
                        n_split += 1
                        nop.engine = (
                            drain_engines[k % len(drain_engines)]
                            if is_drain else inst.engine
                        )
                        nop.sync_info = mybir.SyncInfo(on_wait=[w], on_update=[])
                        il.insert(i, nop)
                        i += 1
                i += 1
    return nc


N_CORES = 8
BATCH = 16
BPC = BATCH // N_CORES  # batches per core
D = 4096  # spatial (64*64)
C = 256   # channels
HID = 512
HEADS = 8

F32 = mybir.dt.float32
F16 = mybir.dt.float16
F8 = mybir.dt.float8e4
F8E3 = mybir.dt.float8e3
E4NP = ml_dtypes.float8_e4m3
E3NP = ml_dtypes.float8_e3m4

# offsets into the packed weight tile w_all [128, 4096] (fp16)
WK_OFF = 0          # wk  [128, 2, 512]
WQ_OFF = 1024       # wq' [128, 2, 512]  (q-scale/8 and Gram-scale/4 folded)
WVT_OFF = 2048      # wvT [128, 4, 256]
WO_OFF = 3072       # wo  [128, 4, 256]

_CACHE = {}


def _build():
    nc = bass.Bass()
    # x twice: d-major (partition = d%128) for G, c-major for Y; each as an
    # fp8 hi/lo residual pair (same bytes as fp16)
    xdh_d = nc.declare_dram_parameter("xdc_hi", [BPC, 128, 32 * C], F8, isOutput=False)
    xdl_d = nc.declare_dram_parameter("xdc_lo", [BPC, 128, 32 * C], F8, isOutput=False)
    xth_d = nc.declare_dram_parameter("xT_hi", [BPC, 128, 2 * D], F8, isOutput=False)
    xtl_d = nc.declare_dram_parameter("xT_lo", [BPC, 128, 2 * D], F8, isOutput=False)
    w_d = nc.declare_dram_parameter("w_all", [128, 4096], F16, isOutput=False)
    # y leaves as fp8e3m4 (4 mantissa bits) at scale 2: ~1.2% quantization,
    # well inside the 2e-2 gate, and it halves the y DMA bytes
    y_d = nc.declare_dram_parameter("y", [BPC, 128, 2 * D], F8E3, isOutput=True)

    with TileContext(nc) as tc:
        with (
            tc.tile_pool(name="consts", bufs=1) as consts,
            tc.tile_pool(name="xdc", bufs=2) as xdc_pool,
            tc.tile_pool(name="xt", bufs=2) as xt_pool,
            tc.tile_pool(name="small", bufs=2) as small_pool,
            tc.tile_pool(name="e64", bufs=2) as e_pool,
            tc.tile_pool(name="stat", bufs=6) as stat_pool,
            tc.tile_pool(name="ysb", bufs=2) as y_pool,
            tc.tile_pool(name="mm", bufs=6, space="PSUM") as mm_pool,
            tc.tile_pool(name="simp", bufs=2, space="PSUM") as sim_pool,
        ):
            w_all = consts.tile([128, 4096], F16, name="w_all")

            # PE p-state warmup: ~5us of dummy matmuls on a zeroed tile so
            # G0's real matmuls start at the full 2.4GHz clock instead of
            # spending their first 3us at the 1.2GHz ramp rate
            warm = consts.tile([128, HID], F16, name="warm")
            nc.gpsimd.memset(warm, 0.0)
            for wi in range(10):
                wps = sim_pool.tile([128, HID], F32, name="warmp", tag="simp")
                nc.tensor.matmul(
                    wps, lhsT=warm[:, 0:128], rhs=warm, start=True, stop=True
                )

            def wk(j):  # [128, 512] c-chunk j
                return w_all[:, WK_OFF + j * HID:WK_OFF + (j + 1) * HID]

            def wq(j, h):  # [128, 64] c-chunk j, head h
                lo = WQ_OFF + j * HID + h * 64
                return w_all[:, lo:lo + 64]

            def wvt(t, m):  # [128, 128]: hid-chunk t, c-half m
                lo = WVT_OFF + t * C + m * 128
                return w_all[:, lo:lo + 128]

            def wo(p):  # [128, 256] rows of head pair p
                lo = WO_OFF + p * C
                return w_all[:, lo:lo + C]

            # ---- x/w tiles + DMA stream (order = issue order) ----
            xdh, xdl, xth, xtl = [], [], [], []
            for b in range(BPC):
                xdh.append(xdc_pool.tile([128, 32, C], F8, name=f"xdh{b}", tag="xdh"))
                xdl.append(xdc_pool.tile([128, 32, C], F8, name=f"xdl{b}", tag="xdl"))
                xth.append(xt_pool.tile([128, 2, D], F8, name=f"xth{b}", tag="xth"))
                xtl.append(xt_pool.tile([128, 2, D], F8, name=f"xtl{b}", tag="xtl"))

            def dma_xdc(b, half):
                ks = slice(half * 16, (half + 1) * 16)
                el = slice(half * 16 * C, (half + 1) * 16 * C)
                nc.sync.dma_start(out=xdh[b][:, ks, :], in_=xdh_d[b, :, el])
                nc.sync.dma_start(out=xdl[b][:, ks, :], in_=xdl_d[b, :, el])

            dma_xdc(0, 0)
            dma_xdc(0, 1)
            nc.sync.dma_start(out=w_all[:, 0:2048], in_=w_d[:, 0:2048])
            dma_xdc(1, 0)
            dma_xdc(1, 1)
            nc.sync.dma_start(out=w_all[:, 2048:4096], in_=w_d[:, 2048:4096])
            for b in range(BPC):
                nc.sync.dma_start(out=xth[b], in_=xth_d[b, :, :])
                nc.sync.dma_start(out=xtl[b], in_=xtl_d[b, :, :])

            def emit_g(b):
                """G = (xh+xl)^T(xh+xl) (3-term) via fp8 DoubleRow over
                d-chunk pairs; PSUM = 4G -> Gsb fp16. Emitted half-by-half
                so the first half's matmuls start under the second's DMA."""
                gps = [
                    mm_pool.tile([128, HID], F32, name=f"gps{m}", tag="mm")
                    for m in range(2)
                ]
                for half in range(2):
                    for ab, (lt, rt) in enumerate(
                        ((xdh, xdh), (xdl, xdh), (xdh, xdl))
                    ):
                        for pp in range(8):
                            p2 = half * 16 + pp * 2
                            ks = slice(p2, p2 + 2)
                            for m in range(2):
                                nc.tensor.matmul(
                                    gps[m][:, 0:C],
                                    lhsT=lt[b][:, ks, m * 128:(m + 1) * 128],
                                    rhs=rt[b][:, ks, :],
                                    start=(half == 0 and ab == 0 and pp == 0),
                                    stop=(half == 1 and ab == 2 and pp == 7),
                                    perf_mode=DR,
                                )
                g_sb = small_pool.tile([128, 2, C], F16, name="gsb", tag="gsb")
                # both copies are on the t2 critical path: use two engines
                nc.vector.tensor_copy(g_sb[:, 0, :], gps[0][:, 0:C])
                nc.scalar.copy(g_sb[:, 1, :], gps[1][:, 0:C])
                return g_sb

            def emit_t2_sim(b, g_sb):
                """t2 = G @ wk -> fp16; sim_h = wq_h^T t2_h -> PSUM.

                G is exactly symmetric (both halves accumulate the same
                products in the same order), so Gsb tile j doubles as the
                [c2-chunk j, c1] stationary operand."""
                t2_sb = small_pool.tile([128, 2, HID], F16, name="t2sb", tag="t2")
                for m in range(2):
                    t2p = mm_pool.tile([128, HID], F32, name="t2p", tag="mm")
                    for j in range(2):
                        nc.tensor.matmul(
                            t2p,
                            lhsT=g_sb[:, j, m * 128:(m + 1) * 128],
                            rhs=wk(j),
                            start=(j == 0),
                            stop=(j == 1),
                        )
                    if m == 0:
                        nc.vector.tensor_copy(t2_sb[:, m, :], t2p)
                    else:
                        nc.scalar.copy(t2_sb[:, m, :], t2p)
                # sim packing: head h=(2p+par) -> rows par*64:+64, cols
                # p*64:+64 of sim_all [128, 256]
                sim_all = sim_pool.tile([128, HID], F32, name="sim", tag="simp")
                nc.vector.memset(sim_all[:, 0:C], 0.0)
                for h in range(HEADS):
                    par, p = h % 2, h // 2
                    rows = slice(par * 64, par * 64 + 64)
                    for j in range(2):
                        nc.tensor.matmul(
                            sim_all[rows, p * 64:(p + 1) * 64],
                            lhsT=wq(j, h),
                            rhs=t2_sb[:, j, h * 64:(h + 1) * 64],
                            start=False,
                            stop=(j == 1),
                            skip_group_check=True,
                        )
                return sim_all

            def emit_softmax_m_weff(b, sim_all):
                """exp (grouped, max-subtracted) -> e64 fp16; 1/s folded
                into wo rows; M via K=64 matmuls; W = wv @ M -> fp8 hi/lo
                pair at scale 256."""
                m_t = stat_pool.tile([128, 4], F32, name="m_t", tag="stat")
                s_t = stat_pool.tile([128, 4], F32, name="s_t", tag="stat")
                r_t = stat_pool.tile([128, 4], F32, name="r_t", tag="stat")
                e64 = e_pool.tile([128, 4, 64], F16, name="e64", tag="e64")
                # neg-max per (partition, head-block): [64,4,64] -> [64,4]
                for par in range(2):
                    rows = slice(par * 64, par * 64 + 64)
                    nc.vector.reduce_max(
                        out=m_t[rows, 0:4],
                        in_=sim_all[rows, 0:C].rearrange("p (g j) -> p g j", g=4),
                        axis=mybir.AxisListType.X,
                        negate=True,
                    )
                for p in range(4):
                    nc.scalar.activation(
                        out=e64[:, p, :],
                        in_=sim_all[:, p * 64:(p + 1) * 64],
                        func=mybir.ActivationFunctionType.Exp,
                        bias=m_t[:, p:p + 1],
                        scale=1.0,
                        accum_out=s_t[:, p:p + 1],
                    )
                nc.vector.reciprocal(r_t, s_t)
                m_sb = small_pool.tile([128, 4, C], F16, name="msb", tag="msb")
                for p in range(4):
                    wop = stat_pool.tile([128, C], F16, name=f"wop{p}", tag="wop")
                    nc.vector.tensor_scalar_mul(wop, wo(p), r_t[:, p:p + 1])
                    mp = mm_pool.tile([128, HID], F32, name="mp", tag="mm")
                    for par in range(2):
                        rows = slice(par * 64, par * 64 + 64)
                        nc.tensor.matmul(
                            mp[rows, 0:C],
                            lhsT=e64[rows, p, :],
                            rhs=wop[rows, :],
                            start=True,
                            stop=True,
                        )
                    nc.any.tensor_copy(m_sb[:, p, :], mp[:, 0:C])
                ws16 = small_pool.tile([128, 2, C], F16, name="ws16", tag="ws16")
                w_hi = small_pool.tile([128, 2, C], F8, name="whi", tag="whi")
                w_lo = small_pool.tile([128, 2, C], F8, name="wlo", tag="wlo")
                for m in range(2):
                    wp = mm_pool.tile([128, HID], F32, name="wp", tag="mm")
                    for t in range(4):
                        nc.tensor.matmul(
                            wp[:, 0:C],
                            lhsT=wvt(t, m),
                            rhs=m_sb[:, t, :],
                            start=(t == 0),
                            stop=(t == 3),
                        )
                    # W * 256 as fp8 hi + residual lo
                    nc.vector.tensor_scalar_mul(ws16[:, m, :], wp[:, 0:C], 256.0)
                    nc.any.tensor_copy(w_hi[:, m, :], ws16[:, m, :])
                    nc.vector.tensor_sub(w_lo[:, m, :], ws16[:, m, :], w_hi[:, m, :])
                return w_hi, w_lo

            def emit_y(b, w_hi, w_lo, yt_sb, d4s):
                """yT = W^T x^T: fp8 DoubleRow, W halves stationary, xT
                moving; PSUM = 4096*y -> fp8e3 at 2y (host divides)."""
                for d4 in d4s:
                    for m in range(2):
                        for dd in range(2):
                            d5 = d4 * 2 + dd
                            cols = slice(d5 * 512, (d5 + 1) * 512)
                            yp = mm_pool.tile([128, HID], F32, name="yp", tag="mm")
                            for ti, (lt, rt) in enumerate(
                                ((w_hi, xth[b]), (w_lo, xth[b]), (w_hi, xtl[b]))
                            ):
                                nc.tensor.matmul(
                                    yp,
                                    lhsT=lt[:, :, m * 128:(m + 1) * 128],
                                    rhs=rt[:, :, cols],
                                    start=(ti == 0),
                                    stop=(ti == 2),
                                    perf_mode=DR,
                                )
                            # 2*y = PSUM * 2^-11, cast to fp8e3
                            if dd == 0:
                                nc.vector.tensor_scalar_mul(
                                    yt_sb[:, m, cols], yp, 2.0 ** -11
                                )
                            else:
                                nc.scalar.mul(yt_sb[:, m, cols], yp, 2.0 ** -11)
                        lo = d4 * 1024
                        nc.sync.dma_start(
                            out=y_d[b, :, m * D + lo:m * D + lo + 1024],
                            in_=yt_sb[:, m, lo:lo + 1024],
                        )

            # ---- schedule: G0 t2/sim0 | G1 (PE busy during softmax0) |
            #      M0 W0 | t2/sim1 Y0... M1 W1 (under Y0 tail) ...Y0 Y1 ----
            yts = [
                y_pool.tile([128, 2, D], F8E3, name=f"ysb{b}", tag="ysb")
                for b in range(BPC)
            ]
            g0 = emit_g(0)
            s0 = emit_t2_sim(0, g0)
            g1 = emit_g(1)
            wh0, wl0 = emit_softmax_m_weff(0, s0)
            s1 = emit_t2_sim(1, g1)
            emit_y(0, wh0, wl0, yts[0], range(0, 3))
            wh1, wl1 = emit_softmax_m_weff(1, s1)
            emit_y(0, wh0, wl0, yts[0], range(3, 4))
            emit_y(1, wh1, wl1, yts[1], range(0, 4))
    return _split_multi_waits(nc)


def _get_nc():
    if "nc" not in _CACHE:
        _CACHE["nc"] = _build()
    return _CACHE["nc"]


def _hilo(x, scale):
    """fp8e4m3 hi + residual lo of x*scale (f32 in, ml_dtypes out)."""
    xs = (x * scale).astype(np.float32)
    hi = xs.astype(E4NP)
    lo = (xs - hi.astype(np.float32)).astype(E4NP)
    return hi, lo


def kernel(x, w_qkv, w_out, b_out, **kw):
    x = np.asarray(x, dtype=np.float32)
    w_qkv = np.asarray(w_qkv, dtype=np.float32)
    w_out = np.asarray(w_out, dtype=np.float32)
    b_out = np.asarray(b_out, dtype=np.float32)

    # fold q-scale/8 and Gram-scale/4 into w_q; pack weights into one
    # [128, 4096] fp16 tile: [wk | wq' | wvT | wo], each c/hid-chunked so
    # partition p holds row t*128+p of the logical matrix in slot t
    wq = (w_qkv[:, :HID] * (64 ** -0.5) * 0.25).astype(np.float16)
    wk = w_qkv[:, HID:2 * HID].astype(np.float16)
    wvT = np.ascontiguousarray(w_qkv[:, 2 * HID:].T).astype(np.float16)
    wo = w_out.astype(np.float16)
    w_all = np.concatenate([
        wk.reshape(2, 128, HID).transpose(1, 0, 2).reshape(128, 2 * HID),
        wq.reshape(2, 128, HID).transpose(1, 0, 2).reshape(128, 2 * HID),
        wvT.reshape(4, 128, C).transpose(1, 0, 2).reshape(128, 4 * C),
        wo.reshape(4, 128, C).transpose(1, 0, 2).reshape(128, 4 * C),
    ], axis=1)
    w_all = np.ascontiguousarray(w_all)

    x4 = x.reshape(BATCH, D, C)
    in_maps = []
    for core in range(N_CORES):
        xb = x4[core * BPC:(core + 1) * BPC]  # [BPC, D, C] f32
        # d-major: partition p <- row k*128+p, free slot k; scale 2
        x_dc = np.ascontiguousarray(
            xb.reshape(BPC, 32, 128, C).transpose(0, 2, 1, 3)
        ).reshape(BPC, 128, 32 * C)
        xdh, xdl = _hilo(x_dc, 2.0)
        # c-major: partition p <- channel t*128+p, free slot t; scale 16
        x_t = np.ascontiguousarray(
            xb.transpose(0, 2, 1).reshape(BPC, 2, 128, D).transpose(0, 2, 1, 3)
        ).reshape(BPC, 128, 2 * D)
        xth, xtl = _hilo(x_t, 16.0)
        in_maps.append({
            "xdc_hi": xdh, "xdc_lo": xdl, "xT_hi": xth, "xT_lo": xtl,
            "w_all": w_all,
        })

    nc = _get_nc()
    res = run_bass_kernel_spmd(nc, in_maps, core_ids=list(range(N_CORES)), **kw)
    # y arrives as 2*yT in fp8e3 [BPC, 128, 2, D]: channel t*128+p, pixel d
    def as_e3(a):
        a = np.asarray(a)
        return a if a.dtype == E3NP else a.view(E3NP)

    yt = np.stack([as_e3(r["y"]) for r in res.results])
    yt = yt.reshape(BATCH, 128, 2, D).transpose(0, 2, 1, 3).reshape(BATCH, C, D)
    y = yt.transpose(0, 2, 1).astype(np.float32) * 0.5 + b_out
    return y.reshape(BATCH, 64, 64, C)


# revision 30
# speedup vs baseline: 3.6384x; 1.0263x over previous
"""Channel-attention Trainium2 Bass kernel, Gram-collapsed + fp8 DoubleRow.

Key identity: this is CHANNEL attention (the softmax mixes the 64 channels
of each head; every pixel is treated identically), so the whole module
collapses to a per-batch 256x256 effective channel-mixing matrix:

    G     = x^T x                     # [256,256] Gram, contracts d=4096
    sim_h = wq_h^T G wk_h             # [64,64] per head  (== (x wq)^T (x wk))
    attn_h = softmax(sim_h)           # denominator folded into wo rows
    M_h   = attn_h^T wo_h             # [64,256]
    W     = wv @ concat_h(M_h)        # [256,256] effective weight
    y     = x @ W (+ b_out)

Only G and y touch the [4096, 256] data; both run as fp8e4m3 DoubleRow
matmuls (0.5 cyc/row, 2 K-tiles per instruction) on hi+lo residual pairs:
a @ b ~= ah@bh + al@bh + ah@bl, where the lo tensors carry the fp8
quantization residual of the hi ones. That keeps fp16-grade accuracy
(end-to-end rel-l2 ~3.4e-3 vs the fp64 oracle) at fp8 speed and the same
DMA bytes as fp16. Scales are powers of two: x_dc*2 (so Gsb=4G stays
under fp16 max), xT*16, W*256; the q-scale/8, G/4 land in w_q host-side
and the 4096x on y divides out on the host.

Softmax denominators never touch e: attn = e/s is realized by scaling
wo's rows by r = 1/s (per-partition tensor_scalar) before the M matmul,
since M's contraction index (attn row i) is exactly wo's row index.

Distribution: data-parallel over batch - 8 cores x 2 batches each, weights
replicated, no collectives. Per-core DMA is the roofline (~13MB at
360GB/s ~= 38us): x twice (d-major for G, c-major for y, 2MB/batch each),
y out fp16 (2MB/batch), weights 1MB. DMA instruction count stays small
(~29/core, ~625ns serialized issue each) and every transfer keeps >=2KB
contiguous per-partition runs for full bandwidth. PE work is ~32k
column-cycles/batch (~27us/core), hidden under the DMA stream.
"""

import numpy as np
import ml_dtypes

import concourse.bass as bass
import concourse.mybir as mybir
from concourse.bass_utils import run_bass_kernel_spmd
from concourse.tile import TileContext

DR = mybir.MatmulPerfMode.DoubleRow


def _split_multi_waits(nc, limit=1):
    """Post-pass: the walrus build in this container rejects instructions
    carrying more than `limit` sync-waits ("Too many sync wait commands" in
    setupSyncWait). Tile attaches up to 3. Hoist the extras onto same-engine
    NoOp instructions inserted immediately before the owner — the engine
    sequencer executes them in order, so the ordering semantics are
    identical."""
    drain_engines = [
        mybir.EngineType.PE,
        mybir.EngineType.DVE,
        mybir.EngineType.Activation,
        mybir.EngineType.Pool,
        mybir.EngineType.SP,
    ]
    n_split = 0
    for f in nc.m.functions:
        for blk in f.blocks:
            il = blk.instructions
            i = 0
            while i < len(il):
                inst = il[i]
                si = inst.sync_info
                waits = list(si.on_wait) if si is not None else []
                if len(waits) > limit:
                    si.on_wait = waits[:limit]
                    is_drain = type(inst).__name__ == "InstDrain"
                    for k, w in enumerate(waits[limit:]):
                        nop = mybir.InstNoOp(
                            name=f"I-waitsplit-{n_split}", ins=[], outs=[]
                        )
                        n_split += 1
                        nop.engine = (
                            drain_engines[k % len(drain_engines)]
                            if is_drain else inst.engine
                        )
                        nop.sync_info = mybir.SyncInfo(on_wait=[w], on_update=[])
                        il.insert(i, nop)
                        i += 1
                i += 1
    return nc


N_CORES = 8
BATCH = 16
BPC = BATCH // N_CORES  # batches per core
D = 4096  # spatial (64*64)
C = 256   # channels
HID = 512
HEADS = 8

F32 = mybir.dt.float32
F16 = mybir.dt.float16
F8 = mybir.dt.float8e4
F8E3 = mybir.dt.float8e3
E4NP = ml_dtypes.float8_e4m3
E3NP = ml_dtypes.float8_e3m4

# offsets into the packed weight tile w_all [128, 4096] (fp16)
WK_OFF = 0          # wk  [128, 2, 512]
WQ_OFF = 1024       # wq' [128, 2, 512]  (q-scale/8 and Gram-scale/4 folded)
WVT_OFF = 2048      # wvT [128, 4, 256]
WO_OFF = 3072       # wo  [128, 4, 256]

_CACHE = {}


def _build():
    nc = bass.Bass()
    # x twice: d-major (partition = d%128) for G, c-major for Y; each as an
    # fp8 hi/lo residual pair (same bytes as fp16)
    xdh_d = nc.declare_dram_parameter("xdc_hi", [BPC, 128, 32 * C], F8, isOutput=False)
    xdl_d = nc.declare_dram_parameter("xdc_lo", [BPC, 128, 32 * C], F8, isOutput=False)
    xth_d = nc.declare_dram_parameter("xT_hi", [BPC, 128, 2 * D], F8, isOutput=False)
    xtl_d = nc.declare_dram_parameter("xT_lo", [BPC, 128, 2 * D], F8, isOutput=False)
    w_d = nc.declare_dram_parameter("w_all", [128, 4096], F16, isOutput=False)
    # y leaves as fp8e3m4 (4 mantissa bits) at scale 2: ~1.2% quantization,
    # well inside the 2e-2 gate, and it halves the y DMA bytes
    y_d = nc.declare_dram_parameter("y", [BPC, 128, 2 * D], F8E3, isOutput=True)

    with TileContext(nc) as tc:
        with (
            tc.tile_pool(name="consts", bufs=1) as consts,
            tc.tile_pool(name="xdc", bufs=2) as xdc_pool,
            tc.tile_pool(name="xt", bufs=2) as xt_pool,
            tc.tile_pool(name="small", bufs=2) as small_pool,
            tc.tile_pool(name="e64", bufs=2) as e_pool,
            tc.tile_pool(name="stat", bufs=6) as stat_pool,
            tc.tile_pool(name="ysb", bufs=2) as y_pool,
            tc.tile_pool(name="mm", bufs=6, space="PSUM") as mm_pool,
            tc.tile_pool(name="simp", bufs=2, space="PSUM") as sim_pool,
        ):
            w_all = consts.tile([128, 4096], F16, name="w_all")

            # PE p-state warmup: ~5us of dummy matmuls on a zeroed tile so
            # G0's real matmuls start at the full 2.4GHz clock instead of
            # spending their first 3us at the 1.2GHz ramp rate
            warm = consts.tile([128, HID], F16, name="warm")
            nc.gpsimd.memset(warm, 0.0)
            for wi in range(10):
                wps = sim_pool.tile([128, HID], F32, name="warmp", tag="simp")
                nc.tensor.matmul(
                    wps, lhsT=warm[:, 0:128], rhs=warm, start=True, stop=True
                )

            def wk(j):  # [128, 512] c-chunk j
                return w_all[:, WK_OFF + j * HID:WK_OFF + (j + 1) * HID]

            def wq(j, h):  # [128, 64] c-chunk j, head h
                lo = WQ_OFF + j * HID + h * 64
                return w_all[:, lo:lo + 64]

            def wvt(t, m):  # [128, 128]: hid-chunk t, c-half m
                lo = WVT_OFF + t * C + m * 128
                return w_all[:, lo:lo + 128]

            def wo(p):  # [128, 256] rows of head pair p
                lo = WO_OFF + p * C
                return w_all[:, lo:lo + C]

            # ---- x/w tiles + DMA stream (order = issue order) ----
            xdh, xdl, xth, xtl = [], [], [], []
            for b in range(BPC):
                xdh.append(xdc_pool.tile([128, 32, C], F8, name=f"xdh{b}", tag="xdh"))
                xdl.append(xdc_pool.tile([128, 32, C], F8, name=f"xdl{b}", tag="xdl"))
                xth.append(xt_pool.tile([128, 2, D], F8, name=f"xth{b}", tag="xth"))
                xtl.append(xt_pool.tile([128, 2, D], F8, name=f"xtl{b}", tag="xtl"))

            def dma_xdc(b, half):
                ks = slice(half * 16, (half + 1) * 16)
                el = slice(half * 16 * C, (half + 1) * 16 * C)
                nc.sync.dma_start(out=xdh[b][:, ks, :], in_=xdh_d[b, :, el])
                nc.sync.dma_start(out=xdl[b][:, ks, :], in_=xdl_d[b, :, el])

            dma_xdc(0, 0)
            dma_xdc(0, 1)
            nc.sync.dma_start(out=w_all[:, 0:2048], in_=w_d[:, 0:2048])
            dma_xdc(1, 0)
            dma_xdc(1, 1)
            nc.sync.dma_start(out=w_all[:, 2048:4096], in_=w_d[:, 2048:4096])
            for b in range(BPC):
                nc.sync.dma_start(out=xth[b], in_=xth_d[b, :, :])
                nc.sync.dma_start(out=xtl[b], in_=xtl_d[b, :, :])

            def emit_g(b):
                """G = (xh+xl)^T(xh+xl) (3-term) via fp8 DoubleRow over
                d-chunk pairs; PSUM = 4G -> Gsb fp16. Emitted half-by-half
                so the first half's matmuls start under the second's DMA."""
                gps = [
                    mm_pool.tile([128, HID], F32, name=f"gps{m}", tag="mm")
                    for m in range(2)
                ]
                for half in range(2):
                    for ab, (lt, rt) in enumerate(
                        ((xdh, xdh), (xdl, xdh), (xdh, xdl))
                    ):
                        for pp in range(8):
                            p2 = half * 16 + pp * 2
                            ks = slice(p2, p2 + 2)
                            for m in range(2):
                                nc.tensor.matmul(
                                    gps[m][:, 0:C],
                                    lhsT=lt[b][:, ks, m * 128:(m + 1) * 128],
                                    rhs=rt[b][:, ks, :],
                                    start=(half == 0 and ab == 0 and pp == 0),
                                    stop=(half == 1 and ab == 2 and pp == 7),
                                    perf_mode=DR,
                                )
                g_sb = small_pool.tile([128, 2, C], F16, name="gsb", tag="gsb")
                # both copies are on the t2 critical path: use two engines
                nc.vector.tensor_copy(g_sb[:, 0, :], gps[0][:, 0:C])
                nc.scalar.copy(g_sb[:, 1, :], gps[1][:, 0:C])
                return g_sb

            def emit_t2_sim(b, g_sb):
                """t2 = G @ wk -> fp16; sim_h = wq_h^T t2_h -> PSUM.

                G is exactly symmetric (both halves accumulate the same
                products in the same order), so Gsb tile j doubles as the
                [c2-chunk j, c1] stationary operand."""
                t2_sb = small_pool.tile([128, 2, HID], F16, name="t2sb", tag="t2")
                for m in range(2):
                    t2p = mm_pool.tile([128, HID], F32, name="t2p", tag="mm")
                    for j in range(2):
                        nc.tensor.matmul(
                            t2p,
                            lhsT=g_sb[:, j, m * 128:(m + 1) * 128],
                            rhs=wk(j),
                            start=(j == 0),
                            stop=(j == 1),
                        )
                    if m == 0:
                        nc.vector.tensor_copy(t2_sb[:, m, :], t2p)
                    else:
                        nc.scalar.copy(t2_sb[:, m, :], t2p)
                # sim packing: head h=(2p+par) -> rows par*64:+64, cols
                # p*64:+64 of sim_all [128, 256]
                sim_all = sim_pool.tile([128, HID], F32, name="sim", tag="simp")
                nc.vector.memset(sim_all[:, 0:C], 0.0)
                for h in range(HEADS):
                    par, p = h % 2, h // 2
                    rows = slice(par * 64, par * 64 + 64)
                    for j in range(2):
                        nc.tensor.matmul(
                            sim_all[rows, p * 64:(p + 1) * 64],
                            lhsT=wq(j, h),
                            rhs=t2_sb[:, j, h * 64:(h + 1) * 64],
                            start=False,
                            stop=(j == 1),
                            skip_group_check=True,
                        )
                return sim_all

            def emit_softmax_stats(b, sim_all):
                """exp (grouped, max-subtracted) -> e64 fp16; 1/s folded
                into wo rows (per-partition scale, since M's contraction
                index is wo's row index). DVE/Act/Pool only - emitted right
                after the sim matmuls so these never queue behind the
                y-phase PSUM drains on the same engines."""
                m_t = stat_pool.tile([128, 4], F32, name="m_t", tag="stat")
                s_t = stat_pool.tile([128, 4], F32, name="s_t", tag="stat")
                r_t = stat_pool.tile([128, 4], F32, name="r_t", tag="stat")
                e64 = e_pool.tile([128, 4, 64], F16, name="e64", tag="e64")
                # neg-max per (partition, head-block): [64,4,64] -> [64,4]
                for par in range(2):
                    rows = slice(par * 64, par * 64 + 64)
                    nc.vector.reduce_max(
                        out=m_t[rows, 0:4],
                        in_=sim_all[rows, 0:C].rearrange("p (g j) -> p g j", g=4),
                        axis=mybir.AxisListType.X,
                        negate=True,
                    )
                for p in range(4):
                    nc.scalar.activation(
                        out=e64[:, p, :],
                        in_=sim_all[:, p * 64:(p + 1) * 64],
                        func=mybir.ActivationFunctionType.Exp,
                        bias=m_t[:, p:p + 1],
                        scale=1.0,
                        accum_out=s_t[:, p:p + 1],
                    )
                nc.vector.reciprocal(r_t, s_t)
                wops = []
                for p in range(4):
                    wop = stat_pool.tile([128, C], F16, name=f"wop{p}", tag="wop")
                    nc.vector.tensor_scalar_mul(wop, wo(p), r_t[:, p:p + 1])
                    wops.append(wop)
                return e64, wops

            def emit_m_weff(b, e64, wops):
                """M via K=64 matmuls; W = wv @ M -> fp8 hi/lo at scale 256."""
                m_sb = small_pool.tile([128, 4, C], F16, name="msb", tag="msb")
                for p in range(4):
                    wop = wops[p]
                    mp = mm_pool.tile([128, HID], F32, name="mp", tag="mm")
                    for par in range(2):
                        rows = slice(par * 64, par * 64 + 64)
                        nc.tensor.matmul(
                            mp[rows, 0:C],
                            lhsT=e64[rows, p, :],
                            rhs=wop[rows, :],
                            start=True,
                            stop=True,
                        )
                    if p % 2 == 0:
                        nc.scalar.copy(m_sb[:, p, :], mp[:, 0:C])
                    else:
                        nc.vector.tensor_copy(m_sb[:, p, :], mp[:, 0:C])
                ws16 = small_pool.tile([128, 2, C], F16, name="ws16", tag="ws16")
                w_hi = small_pool.tile([128, 2, C], F8, name="whi", tag="whi")
                w_lo = small_pool.tile([128, 2, C], F8, name="wlo", tag="wlo")
                for m in range(2):
                    wp = mm_pool.tile([128, HID], F32, name="wp", tag="mm")
                    for t in range(4):
                        nc.tensor.matmul(
                            wp[:, 0:C],
                            lhsT=wvt(t, m),
                            rhs=m_sb[:, t, :],
                            start=(t == 0),
                            stop=(t == 3),
                        )
                    # W * 256 as fp8 hi + residual lo
                    nc.vector.tensor_scalar_mul(ws16[:, m, :], wp[:, 0:C], 256.0)
                    nc.scalar.copy(w_hi[:, m, :], ws16[:, m, :])
                    nc.vector.tensor_sub(w_lo[:, m, :], ws16[:, m, :], w_hi[:, m, :])
                return w_hi, w_lo

            def emit_y(b, w_hi, w_lo, yt_sb, d4s, look=1):
                """yT = W^T x^T: fp8 DoubleRow, W halves stationary, xT
                moving; PSUM = 4096*y -> fp8e3 at 2y (host divides).

                The xtl-dependent third term of each group is emitted one
                group late so the first two terms (which only need xth)
                keep PE busy while the xT_lo DMA is still in flight."""
                groups = [(d4, m, dd) for d4 in d4s for m in range(2)
                          for dd in range(2)]
                yps = {}

                def cols_of(g):
                    d4, m, dd = g
                    return m, slice((d4 * 2 + dd) * 512, (d4 * 2 + dd + 1) * 512)

                def emit_t12(g):
                    m, cols = cols_of(g)
                    yp = mm_pool.tile([128, HID], F32, name="yp", tag="mm")
                    yps[g] = yp
                    for ti, lt in enumerate((w_hi, w_lo)):
                        nc.tensor.matmul(
                            yp,
                            lhsT=lt[:, :, m * 128:(m + 1) * 128],
                            rhs=xth[b][:, :, cols],
                            start=(ti == 0),
                            stop=False,
                            perf_mode=DR,
                        )

                def emit_t3(g):
                    m, cols = cols_of(g)
                    nc.tensor.matmul(
                        yps[g],
                        lhsT=w_hi[:, :, m * 128:(m + 1) * 128],
                        rhs=xtl[b][:, :, cols],
                        start=False,
                        stop=True,
                        perf_mode=DR,
                    )
                    # 2*y = PSUM * 2^-11, cast to fp8e3; rotate engines
                    d4, m_, dd = g
                    eng = (d4 * 4 + m_ * 2 + dd) % 2
                    if eng == 0:
                        nc.scalar.mul(yt_sb[:, m_, cols], yps[g], 2.0 ** -11)
                    else:
                        nc.vector.tensor_scalar_mul(
                            yt_sb[:, m_, cols], yps[g], 2.0 ** -11
                        )
                    if dd == 1 and m_ == 1:
                        for mm_ in range(2):
                            lo = d4 * 1024
                            nc.sync.dma_start(
                                out=y_d[b, :, mm_ * D + lo:mm_ * D + lo + 1024],
                                in_=yt_sb[:, mm_, lo:lo + 1024],
                            )

                for i in range(len(groups) + look):
                    if i < len(groups):
                        emit_t12(groups[i])
                    if i >= look:
                        emit_t3(groups[i - look])

            # ---- schedule: G0 t2/sim0 | G1 (PE busy during softmax0) |
            #      M0 W0 | t2/sim1 Y0... M1 W1 (under Y0 tail) ...Y0 Y1 ----
            yts = [
                y_pool.tile([128, 2, D], F8E3, name=f"ysb{b}", tag="ysb")
                for b in range(BPC)
            ]
            g0 = emit_g(0)
            s0 = emit_t2_sim(0, g0)
            st0 = emit_softmax_stats(0, s0)
            g1 = emit_g(1)
            wh0, wl0 = emit_m_weff(0, *st0)
            s1 = emit_t2_sim(1, g1)
            st1 = emit_softmax_stats(1, s1)
            emit_y(0, wh0, wl0, yts[0], range(0, 2), look=1)
            wh1, wl1 = emit_m_weff(1, *st1)
            emit_y(0, wh0, wl0, yts[0], range(2, 4), look=1)
            emit_y(1, wh1, wl1, yts[1], range(0, 4), look=2)
    return _split_multi_waits(nc)


def _get_nc():
    if "nc" not in _CACHE:
        _CACHE["nc"] = _build()
    return _CACHE["nc"]


def _hilo(x, scale):
    """fp8e4m3 hi + residual lo of x*scale (f32 in, ml_dtypes out)."""
    xs = (x * scale).astype(np.float32)
    hi = xs.astype(E4NP)
    lo = (xs - hi.astype(np.float32)).astype(E4NP)
    return hi, lo


def kernel(x, w_qkv, w_out, b_out, **kw):
    x = np.asarray(x, dtype=np.float32)
    w_qkv = np.asarray(w_qkv, dtype=np.float32)
    w_out = np.asarray(w_out, dtype=np.float32)
    b_out = np.asarray(b_out, dtype=np.float32)

    # fold q-scale/8 and Gram-scale/4 into w_q; pack weights into one
    # [128, 4096] fp16 tile: [wk | wq' | wvT | wo], each c/hid-chunked so
    # partition p holds row t*128+p of the logical matrix in slot t
    wq = (w_qkv[:, :HID] * (64 ** -0.5) * 0.25).astype(np.float16)
    wk = w_qkv[:, HID:2 * HID].astype(np.float16)
    wvT = np.ascontiguousarray(w_qkv[:, 2 * HID:].T).astype(np.float16)
    wo = w_out.astype(np.float16)
    w_all = np.concatenate([
        wk.reshape(2, 128, HID).transpose(1, 0, 2).reshape(128, 2 * HID),
        wq.reshape(2, 128, HID).transpose(1, 0, 2).reshape(128, 2 * HID),
        wvT.reshape(4, 128, C).transpose(1, 0, 2).reshape(128, 4 * C),
        wo.reshape(4, 128, C).transpose(1, 0, 2).reshape(128, 4 * C),
    ], axis=1)
    w_all = np.ascontiguousarray(w_all)

    x4 = x.reshape(BATCH, D, C)
    in_maps = []
    for core in range(N_CORES):
        xb = x4[core * BPC:(core + 1) * BPC]  # [BPC, D, C] f32
        # d-major: partition p <- row k*128+p, free slot k; scale 2
        x_dc = np.ascontiguousarray(
            xb.reshape(BPC, 32, 128, C).transpose(0, 2, 1, 3)
        ).reshape(BPC, 128, 32 * C)
        xdh, xdl = _hilo(x_dc, 2.0)
        # c-major: partition p <- channel t*128+p, free slot t; scale 16
        x_t = np.ascontiguousarray(
            xb.transpose(0, 2, 1).reshape(BPC, 2, 128, D).transpose(0, 2, 1, 3)
        ).reshape(BPC, 128, 2 * D)
        xth, xtl = _hilo(x_t, 16.0)
        in_maps.append({
            "xdc_hi": xdh, "xdc_lo": xdl, "xT_hi": xth, "xT_lo": xtl,
            "w_all": w_all,
        })

    nc = _get_nc()
    res = run_bass_kernel_spmd(nc, in_maps, core_ids=list(range(N_CORES)), **kw)
    # y arrives as 2*yT in fp8e3 [BPC, 128, 2, D]: channel t*128+p, pixel d
    def as_e3(a):
        a = np.asarray(a)
        return a if a.dtype == E3NP else a.view(E3NP)

    yt = np.stack([as_e3(r["y"]) for r in res.results])
    yt = yt.reshape(BATCH, 128, 2, D).transpose(0, 2, 1, 3).reshape(BATCH, C, D)
    y = yt.transpose(0, 2, 1).astype(np.float32) * 0.5 + b_out
    return y.reshape(BATCH, 64, 64, C)


# revision 41
# speedup vs baseline: 3.6688x; 1.0083x over previous
"""Channel-attention Trainium2 Bass kernel, Gram-collapsed + fp8 DoubleRow.

Key identity: this is CHANNEL attention (the softmax mixes the 64 channels
of each head; every pixel is treated identically), so the whole module
collapses to a per-batch 256x256 effective channel-mixing matrix:

    G     = x^T x                     # [256,256] Gram, contracts d=4096
    sim_h = wq_h^T G wk_h             # [64,64] per head  (== (x wq)^T (x wk))
    attn_h = softmax(sim_h)           # denominator folded into wo rows
    M_h   = attn_h^T wo_h             # [64,256]
    W     = wv @ concat_h(M_h)        # [256,256] effective weight
    y     = x @ W (+ b_out)

Only G and y touch the [4096, 256] data; both run as fp8e4m3 DoubleRow
matmuls (0.5 cyc/row, 2 K-tiles per instruction) on hi+lo residual pairs:
a @ b ~= ah@bh + al@bh + ah@bl, where the lo tensors carry the fp8
quantization residual of the hi ones. That keeps fp16-grade accuracy
(end-to-end rel-l2 ~3.4e-3 vs the fp64 oracle) at fp8 speed and the same
DMA bytes as fp16. Scales are powers of two: x_dc*2 (so Gsb=4G stays
under fp16 max), xT*16, W*256; the q-scale/8, G/4 land in w_q host-side
and the 4096x on y divides out on the host.

Softmax denominators never touch e: attn = e/s is realized by scaling
wo's rows by r = 1/s (per-partition tensor_scalar) before the M matmul,
since M's contraction index (attn row i) is exactly wo's row index.

Distribution: data-parallel over batch - 8 cores x 2 batches each, weights
replicated, no collectives. Per-core DMA is the roofline (~13MB at
360GB/s ~= 38us): x twice (d-major for G, c-major for y, 2MB/batch each),
y out fp16 (2MB/batch), weights 1MB. DMA instruction count stays small
(~29/core, ~625ns serialized issue each) and every transfer keeps >=2KB
contiguous per-partition runs for full bandwidth. PE work is ~32k
column-cycles/batch (~27us/core), hidden under the DMA stream.
"""

import numpy as np
import ml_dtypes

import concourse.bass as bass
import concourse.mybir as mybir
from concourse.bass_utils import run_bass_kernel_spmd
from concourse.tile import TileContext

DR = mybir.MatmulPerfMode.DoubleRow


def _split_multi_waits(nc, limit=1):
    """Post-pass: the walrus build in this container rejects instructions
    carrying more than `limit` sync-waits ("Too many sync wait commands" in
    setupSyncWait). Tile attaches up to 3. Hoist the extras onto same-engine
    NoOp instructions inserted immediately before the owner — the engine
    sequencer executes them in order, so the ordering semantics are
    identical."""
    drain_engines = [
        mybir.EngineType.PE,
        mybir.EngineType.DVE,
        mybir.EngineType.Activation,
        mybir.EngineType.Pool,
        mybir.EngineType.SP,
    ]
    n_split = 0
    for f in nc.m.functions:
        for blk in f.blocks:
            il = blk.instructions
            i = 0
            while i < len(il):
                inst = il[i]
                si = inst.sync_info
                waits = list(si.on_wait) if si is not None else []
                if len(waits) > limit:
                    si.on_wait = waits[:limit]
                    is_drain = type(inst).__name__ == "InstDrain"
                    for k, w in enumerate(waits[limit:]):
                        nop = mybir.InstNoOp(
                            name=f"I-waitsplit-{n_split}", ins=[], outs=[]
                        )
                        n_split += 1
                        nop.engine = (
                            drain_engines[k % len(drain_engines)]
                            if is_drain else inst.engine
                        )
                        nop.sync_info = mybir.SyncInfo(on_wait=[w], on_update=[])
                        il.insert(i, nop)
                        i += 1
                i += 1
    return nc


N_CORES = 8
BATCH = 16
BPC = BATCH // N_CORES  # batches per core
D = 4096  # spatial (64*64)
C = 256   # channels
HID = 512
HEADS = 8

F32 = mybir.dt.float32
F16 = mybir.dt.float16
F8 = mybir.dt.float8e4
F8E3 = mybir.dt.float8e3
E4NP = ml_dtypes.float8_e4m3
E3NP = ml_dtypes.float8_e3m4

# offsets into the packed weight tile w_all [128, 4096] (fp16)
WK_OFF = 0          # wk  [128, 2, 512]
WQ_OFF = 1024       # wq' [128, 2, 512]  (q-scale/8 and Gram-scale/4 folded)
WVT_OFF = 2048      # wvT [128, 4, 256]
WO_OFF = 3072       # wo  [128, 4, 256]

_CACHE = {}


def _build():
    nc = bass.Bass()
    # x twice: d-major (partition = d%128) for G, c-major for Y; each as an
    # fp8 hi/lo residual pair (same bytes as fp16)
    xdh_d = nc.declare_dram_parameter("xdc_hi", [BPC, 128, 32 * C], F8, isOutput=False)
    xdl_d = nc.declare_dram_parameter("xdc_lo", [BPC, 128, 32 * C], F8, isOutput=False)
    xth_d = nc.declare_dram_parameter("xT_hi", [BPC, 128, 2 * D], F8, isOutput=False)
    xtl_d = nc.declare_dram_parameter("xT_lo", [BPC, 128, 2 * D], F8, isOutput=False)
    w_d = nc.declare_dram_parameter("w_all", [128, 4096], F16, isOutput=False)
    # y leaves as fp8e3m4 (4 mantissa bits) at scale 2: ~1.2% quantization,
    # well inside the 2e-2 gate, and it halves the y DMA bytes
    y_d = nc.declare_dram_parameter("y", [BPC, 128, 2 * D], F8E3, isOutput=True)

    with TileContext(nc) as tc:
        with (
            tc.tile_pool(name="consts", bufs=1) as consts,
            tc.tile_pool(name="xdc", bufs=2) as xdc_pool,
            tc.tile_pool(name="xt", bufs=2) as xt_pool,
            tc.tile_pool(name="small", bufs=2) as small_pool,
            tc.tile_pool(name="e64", bufs=2) as e_pool,
            tc.tile_pool(name="stat", bufs=6) as stat_pool,
            tc.tile_pool(name="ysb", bufs=2) as y_pool,
            tc.tile_pool(name="mm", bufs=6, space="PSUM") as mm_pool,
            tc.tile_pool(name="simp", bufs=2, space="PSUM") as sim_pool,
        ):
            w_all = consts.tile([128, 4096], F16, name="w_all")

            # PE p-state warmup: ~5us of dummy matmuls on a zeroed tile so
            # G0's real matmuls start at the full 2.4GHz clock instead of
            # spending their first 3us at the 1.2GHz ramp rate
            warm = consts.tile([128, HID], F16, name="warm")
            nc.gpsimd.memset(warm, 0.0)
            for wi in range(10):
                wps = sim_pool.tile([128, HID], F32, name="warmp", tag="simp")
                nc.tensor.matmul(
                    wps, lhsT=warm[:, 0:128], rhs=warm, start=True, stop=True
                )

            def wk(j):  # [128, 512] c-chunk j
                return w_all[:, WK_OFF + j * HID:WK_OFF + (j + 1) * HID]

            def wq(j, h):  # [128, 64] c-chunk j, head h
                lo = WQ_OFF + j * HID + h * 64
                return w_all[:, lo:lo + 64]

            def wvt(t, m):  # [128, 128]: hid-chunk t, c-half m
                lo = WVT_OFF + t * C + m * 128
                return w_all[:, lo:lo + 128]

            def wo(p):  # [128, 256] rows of head pair p
                lo = WO_OFF + p * C
                return w_all[:, lo:lo + C]

            # ---- x/w tiles + DMA stream (order = issue order) ----
            xdh, xdl, xth, xtl = [], [], [], []
            for b in range(BPC):
                xdh.append(xdc_pool.tile([128, 32, C], F8, name=f"xdh{b}", tag="xdh"))
                xdl.append(xdc_pool.tile([128, 32, C], F8, name=f"xdl{b}", tag="xdl"))
                xth.append(xt_pool.tile([128, 2, D], F8, name=f"xth{b}", tag="xth"))
                xtl.append(xt_pool.tile([128, 2, D], F8, name=f"xtl{b}", tag="xtl"))

            def dma_xdc(b, half):
                ks = slice(half * 16, (half + 1) * 16)
                el = slice(half * 16 * C, (half + 1) * 16 * C)
                nc.sync.dma_start(out=xdh[b][:, ks, :], in_=xdh_d[b, :, el])
                nc.sync.dma_start(out=xdl[b][:, ks, :], in_=xdl_d[b, :, el])

            dma_xdc(0, 0)
            dma_xdc(0, 1)
            nc.sync.dma_start(out=w_all[:, 0:2048], in_=w_d[:, 0:2048])
            dma_xdc(1, 0)
            dma_xdc(1, 1)
            nc.sync.dma_start(out=w_all[:, 2048:4096], in_=w_d[:, 2048:4096])
            for b in range(BPC):
                nc.sync.dma_start(out=xth[b], in_=xth_d[b, :, :])
                nc.sync.dma_start(out=xtl[b], in_=xtl_d[b, :, :])

            def emit_g(b):
                """G = (xh+xl)^T(xh+xl) (3-term) via fp8 DoubleRow over
                d-chunk pairs; PSUM = 4G -> Gsb fp16. Emitted half-by-half
                so the first half's matmuls start under the second's DMA."""
                gps = [
                    mm_pool.tile([128, HID], F32, name=f"gps{m}", tag="mm")
                    for m in range(2)
                ]
                for half in range(2):
                    for ab, (lt, rt) in enumerate(
                        ((xdh, xdh), (xdl, xdh), (xdh, xdl))
                    ):
                        for pp in range(8):
                            p2 = half * 16 + pp * 2
                            ks = slice(p2, p2 + 2)
                            for m in range(2):
                                nc.tensor.matmul(
                                    gps[m][:, 0:C],
                                    lhsT=lt[b][:, ks, m * 128:(m + 1) * 128],
                                    rhs=rt[b][:, ks, :],
                                    start=(half == 0 and ab == 0 and pp == 0),
                                    stop=(half == 1 and ab == 2 and pp == 7),
                                    perf_mode=DR,
                                )
                g_sb = small_pool.tile([128, 2, C], F16, name="gsb", tag="gsb")
                # both copies are on the t2 critical path: use two engines
                nc.vector.tensor_copy(g_sb[:, 0, :], gps[0][:, 0:C])
                nc.scalar.copy(g_sb[:, 1, :], gps[1][:, 0:C])
                return g_sb

            def emit_t2_sim(b, g_sb):
                """t2 = G @ wk -> fp16; sim_h = wq_h^T t2_h -> PSUM.

                G is exactly symmetric (both halves accumulate the same
                products in the same order), so Gsb tile j doubles as the
                [c2-chunk j, c1] stationary operand."""
                t2_sb = small_pool.tile([128, 2, HID], F16, name="t2sb", tag="t2")
                for m in range(2):
                    t2p = mm_pool.tile([128, HID], F32, name="t2p", tag="mm")
                    for j in range(2):
                        nc.tensor.matmul(
                            t2p,
                            lhsT=g_sb[:, j, m * 128:(m + 1) * 128],
                            rhs=wk(j),
                            start=(j == 0),
                            stop=(j == 1),
                        )
                    if m == 0:
                        nc.vector.tensor_copy(t2_sb[:, m, :], t2p)
                    else:
                        nc.scalar.copy(t2_sb[:, m, :], t2p)
                # sim packing: head h=(2p+par) -> rows par*64:+64, cols
                # p*64:+64 of sim_all [128, 256]
                sim_all = sim_pool.tile([128, HID], F32, name="sim", tag="simp")
                nc.vector.memset(sim_all[:, 0:C], 0.0)
                for h in range(HEADS):
                    par, p = h % 2, h // 2
                    rows = slice(par * 64, par * 64 + 64)
                    for j in range(2):
                        nc.tensor.matmul(
                            sim_all[rows, p * 64:(p + 1) * 64],
                            lhsT=wq(j, h),
                            rhs=t2_sb[:, j, h * 64:(h + 1) * 64],
                            start=False,
                            stop=(j == 1),
                            skip_group_check=True,
                        )
                return sim_all

            def emit_softmax_stats(b, sim_all):
                """exp (grouped, max-subtracted) -> e64 fp16; 1/s folded
                into wo rows (per-partition scale, since M's contraction
                index is wo's row index). DVE/Act/Pool only - emitted right
                after the sim matmuls so these never queue behind the
                y-phase PSUM drains on the same engines."""
                m_t = stat_pool.tile([128, 4], F32, name="m_t", tag="stat")
                s_t = stat_pool.tile([128, 4], F32, name="s_t", tag="stat")
                r_t = stat_pool.tile([128, 4], F32, name="r_t", tag="stat")
                e64 = e_pool.tile([128, 4, 64], F16, name="e64", tag="e64")
                # neg-max per (partition, head-block): [64,4,64] -> [64,4]
                for par in range(2):
                    rows = slice(par * 64, par * 64 + 64)
                    nc.vector.reduce_max(
                        out=m_t[rows, 0:4],
                        in_=sim_all[rows, 0:C].rearrange("p (g j) -> p g j", g=4),
                        axis=mybir.AxisListType.X,
                        negate=True,
                    )
                for p in range(4):
                    nc.scalar.activation(
                        out=e64[:, p, :],
                        in_=sim_all[:, p * 64:(p + 1) * 64],
                        func=mybir.ActivationFunctionType.Exp,
                        bias=m_t[:, p:p + 1],
                        scale=1.0,
                        accum_out=s_t[:, p:p + 1],
                    )
                nc.vector.reciprocal(r_t, s_t)
                wops = []
                for p in range(4):
                    wop = stat_pool.tile([128, C], F16, name=f"wop{p}", tag="wop")
                    nc.vector.tensor_scalar_mul(wop, wo(p), r_t[:, p:p + 1])
                    wops.append(wop)
                return e64, wops

            def emit_m_weff(b, e64, wops):
                """M via K=64 matmuls; W = wv @ M -> fp8 hi/lo at scale 256."""
                m_sb = small_pool.tile([128, 4, C], F16, name="msb", tag="msb")
                for p in range(4):
                    wop = wops[p]
                    mp = mm_pool.tile([128, HID], F32, name="mp", tag="mm")
                    for par in range(2):
                        rows = slice(par * 64, par * 64 + 64)
                        nc.tensor.matmul(
                            mp[rows, 0:C],
                            lhsT=e64[rows, p, :],
                            rhs=wop[rows, :],
                            start=True,
                            stop=True,
                        )
                    if p % 2 == 0:
                        nc.scalar.copy(m_sb[:, p, :], mp[:, 0:C])
                    else:
                        nc.vector.tensor_copy(m_sb[:, p, :], mp[:, 0:C])
                ws16 = small_pool.tile([128, 2, C], F16, name="ws16", tag="ws16")
                w_hi = small_pool.tile([128, 2, C], F8, name="whi", tag="whi")
                w_lo = small_pool.tile([128, 2, C], F8, name="wlo", tag="wlo")
                for m in range(2):
                    wp = mm_pool.tile([128, HID], F32, name="wp", tag="mm")
                    for t in range(4):
                        nc.tensor.matmul(
                            wp[:, 0:C],
                            lhsT=wvt(t, m),
                            rhs=m_sb[:, t, :],
                            start=(t == 0),
                            stop=(t == 3),
                        )
                    # W * 256 as fp8 hi + residual lo
                    nc.vector.tensor_scalar_mul(ws16[:, m, :], wp[:, 0:C], 256.0)
                    nc.scalar.copy(w_hi[:, m, :], ws16[:, m, :])
                    nc.vector.tensor_sub(w_lo[:, m, :], ws16[:, m, :], w_hi[:, m, :])
                return w_hi, w_lo

            def emit_y(b, w_hi, w_lo, yt_sb, d4s, look=1):
                """yT = W^T x^T: fp8 DoubleRow, W halves stationary, xT
                moving; PSUM = 4096*y -> fp8e3 at 2y (host divides).

                The xtl-dependent third term of each group is emitted one
                group late so the first two terms (which only need xth)
                keep PE busy while the xT_lo DMA is still in flight."""
                groups = [(d4, m, dd) for d4 in d4s for m in range(2)
                          for dd in range(2)]
                yps = {}

                def cols_of(g):
                    d4, m, dd = g
                    return m, slice((d4 * 2 + dd) * 512, (d4 * 2 + dd + 1) * 512)

                def emit_t12(g):
                    m, cols = cols_of(g)
                    yp = mm_pool.tile([128, HID], F32, name="yp", tag="mm")
                    yps[g] = yp
                    for ti, lt in enumerate((w_hi, w_lo)):
                        nc.tensor.matmul(
                            yp,
                            lhsT=lt[:, :, m * 128:(m + 1) * 128],
                            rhs=xth[b][:, :, cols],
                            start=(ti == 0),
                            stop=False,
                            perf_mode=DR,
                        )

                def emit_t3(g):
                    m, cols = cols_of(g)
                    nc.tensor.matmul(
                        yps[g],
                        lhsT=w_hi[:, :, m * 128:(m + 1) * 128],
                        rhs=xtl[b][:, :, cols],
                        start=False,
                        stop=True,
                        perf_mode=DR,
                    )
                    # 2*y = PSUM * 2^-11, cast to fp8e3; rotate engines
                    d4, m_, dd = g
                    eng = (d4 * 4 + m_ * 2 + dd) % 2
                    if eng == 0:
                        nc.scalar.mul(yt_sb[:, m_, cols], yps[g], 2.0 ** -11)
                    else:
                        nc.vector.tensor_scalar_mul(
                            yt_sb[:, m_, cols], yps[g], 2.0 ** -11
                        )
                    if dd == 1:
                        lo = d4 * 1024
                        nc.sync.dma_start(
                            out=y_d[b, :, m_ * D + lo:m_ * D + lo + 1024],
                            in_=yt_sb[:, m_, lo:lo + 1024],
                        )

                for i in range(len(groups) + look):
                    if i < len(groups):
                        emit_t12(groups[i])
                    if i >= look:
                        emit_t3(groups[i - look])

            # ---- schedule: G0 t2/sim0 | G1 (PE busy during softmax0) |
            #      M0 W0 | t2/sim1 Y0... M1 W1 (under Y0 tail) ...Y0 Y1 ----
            yts = [
                y_pool.tile([128, 2, D], F8E3, name=f"ysb{b}", tag="ysb")
                for b in range(BPC)
            ]
            g0 = emit_g(0)
            s0 = emit_t2_sim(0, g0)
            st0 = emit_softmax_stats(0, s0)
            g1 = emit_g(1)
            s1 = emit_t2_sim(1, g1)
            st1 = emit_softmax_stats(1, s1)
            wh0, wl0 = emit_m_weff(0, *st0)
            emit_y(0, wh0, wl0, yts[0], range(0, 2), look=1)
            wh1, wl1 = emit_m_weff(1, *st1)
            emit_y(0, wh0, wl0, yts[0], range(2, 4), look=1)
            emit_y(1, wh1, wl1, yts[1], range(0, 4), look=2)
    return _split_multi_waits(nc)


def _get_nc():
    if "nc" not in _CACHE:
        _CACHE["nc"] = _build()
    return _CACHE["nc"]


def _hilo(x, scale):
    """fp8e4m3 hi + residual lo of x*scale (f32 in, ml_dtypes out)."""
    xs = (x * scale).astype(np.float32)
    hi = xs.astype(E4NP)
    lo = (xs - hi.astype(np.float32)).astype(E4NP)
    return hi, lo


def kernel(x, w_qkv, w_out, b_out, **kw):
    x = np.asarray(x, dtype=np.float32)
    w_qkv = np.asarray(w_qkv, dtype=np.float32)
    w_out = np.asarray(w_out, dtype=np.float32)
    b_out = np.asarray(b_out, dtype=np.float32)

    # fold q-scale/8 and Gram-scale/4 into w_q; pack weights into one
    # [128, 4096] fp16 tile: [wk | wq' | wvT | wo], each c/hid-chunked so
    # partition p holds row t*128+p of the logical matrix in slot t
    wq = (w_qkv[:, :HID] * (64 ** -0.5) * 0.25).astype(np.float16)
    wk = w_qkv[:, HID:2 * HID].astype(np.float16)
    wvT = np.ascontiguousarray(w_qkv[:, 2 * HID:].T).astype(np.float16)
    wo = w_out.astype(np.float16)
    w_all = np.concatenate([
        wk.reshape(2, 128, HID).transpose(1, 0, 2).reshape(128, 2 * HID),
        wq.reshape(2, 128, HID).transpose(1, 0, 2).reshape(128, 2 * HID),
        wvT.reshape(4, 128, C).transpose(1, 0, 2).reshape(128, 4 * C),
        wo.reshape(4, 128, C).transpose(1, 0, 2).reshape(128, 4 * C),
    ], axis=1)
    w_all = np.ascontiguousarray(w_all)

    x4 = x.reshape(BATCH, D, C)
    in_maps = []
    for core in range(N_CORES):
        xb = x4[core * BPC:(core + 1) * BPC]  # [BPC, D, C] f32
        # d-major: partition p <- row k*128+p, free slot k; scale 2
        x_dc = np.ascontiguousarray(
            xb.reshape(BPC, 32, 128, C).transpose(0, 2, 1, 3)
        ).reshape(BPC, 128, 32 * C)
        xdh, xdl = _hilo(x_dc, 2.0)
        # c-major: partition p <- channel t*128+p, free slot t; scale 16
        x_t = np.ascontiguousarray(
            xb.transpose(0, 2, 1).reshape(BPC, 2, 128, D).transpose(0, 2, 1, 3)
        ).reshape(BPC, 128, 2 * D)
        xth, xtl = _hilo(x_t, 16.0)
        in_maps.append({
            "xdc_hi": xdh, "xdc_lo": xdl, "xT_hi": xth, "xT_lo": xtl,
            "w_all": w_all,
        })

    nc = _get_nc()
    res = run_bass_kernel_spmd(nc, in_maps, core_ids=list(range(N_CORES)), **kw)
    # y arrives as 2*yT in fp8e3 [BPC, 128, 2, D]: channel t*128+p, pixel d
    def as_e3(a):
        a = np.asarray(a)
        return a if a.dtype == E3NP else a.view(E3NP)

    yt = np.stack([as_e3(r["y"]) for r in res.results])
    yt = yt.reshape(BATCH, 128, 2, D).transpose(0, 2, 1, 3).reshape(BATCH, C, D)
    y = yt.transpose(0, 2, 1).astype(np.float32) * 0.5 + b_out
    return y.reshape(BATCH, 64, 64, C)
